# revision 31
# baseline (speedup 1.0000x reference)
"""Trainium2 Bass kernel for nn_IntegerCifar10Net (quantized VGG-ish CNN).

Data parallel over 8 NeuronCores, B=64 images/core.

v2: DoubleRow fp8 matmuls contract two conv taps per moving stream
(9 taps -> 5 DR matmuls), all matmuls full 128-column, activations in
batch-major padded SBUF layout [ci, y, x, b] fp8 integers {0..7} (=7*v).

Layer schemes:
  L1 : exact 3-plane bf16 im2col (K=81), co=64; even/odd output rows go to
       PSUM partitions 0-63 / 64-127 via column-group alternation so the
       scale/round pipeline runs on all 128 lanes.
  L2 : "halves" trick - PSUM partitions 0-63 = left 16 output cols,
       64-127 = right 16 (upper SBUF partitions hold an x+16 shifted copy);
       block-diagonal [W 0; 0 W] weights, 5 DR matmuls per bank.
  L3 : upper partitions hold x+1 shifted copy (dx-packing): 6 virtual taps
       -> 3 DR matmuls per bank, co=128.
  L4-L6: plain 9 (or 2x9) taps -> 5 (10) DR matmuls per bank.
  FC : as before (N=64 matmuls are dispatch-floor bound and cheap).

Quant chain per bank: ScalarE activation (relu, per-channel scale/bias),
VectorE max-pool (free-dim pairs), RNE round via +/-1.5*2^23 magic, clamp,
cast to fp8. Shift copies (x+16 / x+1) are chunked SBUF->SBUF DMAs
interleaved with compute rows so they stay off the critical path.
"""

import sys
import numpy as np

sys.path.insert(0, "/opt/trn_rl_repo")

import ml_dtypes

N_CORES = 8
B = 64  # images per core
MAGIC = 12582912.0  # 1.5 * 2^23 : RNE rounding magic for |v| < 2^22
L1_ALT = True  # alternate PSUM column groups for even/odd L1 rows

# tap pair schedule for 3x3 convs: pairs of taps t=(dy,dx) row-major,
# 10th tap is zero-weight padding with moving delta -B (always in bounds)
TAPS = [(dy, dx) for dy in range(3) for dx in range(3)]


def _pair_deltas(W):
    """Moving-AP element deltas between the two taps of each DR pair."""
    ds = []
    for p in range(4):
        (dya, dxa), (dyb, dxb) = TAPS[2 * p], TAPS[2 * p + 1]
        ds.append(((dyb - dya) * W + (dxb - dxa)) * B)
    ds.append(-B)  # pad pair: (t8, zero-weight tap at x-1)
    return ds


# ----------------------------------------------------------------------------
# Host-side packing
# ----------------------------------------------------------------------------

def _qint(w):
    """round(clip(w,-1,1)*7) as float32 integers, matching jax fp32 chain."""
    w = np.asarray(w, np.float32)
    return np.round(np.clip(w, -1.0, 1.0) * np.float32(7.0)).astype(np.float32)


def _scale_bias(g, b, denom):
    # z7 = conv_int * (7*g/denom) + 7*b, constants in f64 then rounded to f32
    s = (7.0 * np.asarray(g, np.float64) / denom).astype(np.float32)
    bt = (7.0 * np.asarray(b, np.float64)).astype(np.float32)
    return np.ascontiguousarray(np.stack([s, bt], axis=1))  # [co, 2] f32


def _im2col_bf16x3(x):
    """x [B,3,32,32] f32 -> [81, 32, 32, B] bf16 (hi/mid/lo x 27 rows)."""
    Bc = x.shape[0]
    xp = np.zeros((Bc, 3, 34, 34), np.float32)
    xp[:, :, 1:33, 1:33] = x
    planes = np.empty((27, 32, 32, Bc), np.float32)
    k = 0
    for ci in range(3):
        for dy in range(3):
            for dx in range(3):
                planes[k] = np.transpose(xp[:, ci, dy:dy + 32, dx:dx + 32],
                                         (1, 2, 0))
                k += 1
    hi = planes.astype(ml_dtypes.bfloat16)
    r1 = planes - hi.astype(np.float32)
    mid = r1.astype(ml_dtypes.bfloat16)
    lo = (r1 - mid.astype(np.float32)).astype(ml_dtypes.bfloat16)
    return np.ascontiguousarray(np.concatenate([hi, mid, lo], axis=0))


def host_pack(inputs):
    """Build the per-core DRAM input dicts (weights replicated)."""
    f8 = ml_dtypes.float8_e4m3
    wc = {}
    # L1 weights: [64,3,3,3] -> lhsT [27,64], tripled to [81,64] bf16
    t = np.transpose(_qint(inputs["w1"]), (1, 2, 3, 0)).reshape(27, 64)
    wc["w1sb"] = np.ascontiguousarray(
        np.concatenate([t, t, t], axis=0).astype(ml_dtypes.bfloat16))
    sb = _scale_bias(inputs["g1"], inputs["b1"], 7.0)  # [64, 2]
    wc["sb1"] = np.ascontiguousarray(
        np.concatenate([sb, sb], axis=0).reshape(128, 1, 2))
    # L2: halves block-diagonal [128, 10, 128]
    wq = _qint(inputs["w2"])  # [64co, 64ci, 3, 3]
    w2p = np.zeros((128, 10, 128), np.float32)
    for ti, (dy, dx) in enumerate(TAPS):
        blk = wq[:, :, dy, dx].T  # [ci, co]
        w2p[0:64, ti, 0:64] = blk
        w2p[64:128, ti, 64:128] = blk
    wc["w2sb"] = np.ascontiguousarray(w2p.astype(f8))
    sb = _scale_bias(inputs["g2"], inputs["b2"], 49.0)
    wc["sb2"] = np.ascontiguousarray(
        np.concatenate([sb, sb], axis=0).reshape(128, 1, 2))
    # L3: dx-packed virtual taps [128, 6, 128]
    wq = _qint(inputs["w3"])  # [128co, 64ci, 3, 3]
    w3p = np.zeros((128, 6, 128), np.float32)
    for dy in range(3):
        for oi, o in enumerate((0, 2)):
            v = 2 * dy + oi
            w3p[0:64, v, :] = wq[:, :, dy, o].T
            if o == 0:
                w3p[64:128, v, :] = wq[:, :, dy, 1].T
    wc["w3sb"] = np.ascontiguousarray(w3p.astype(f8))
    wc["sb3"] = np.ascontiguousarray(
        _scale_bias(inputs["g3"], inputs["b3"], 49.0).reshape(128, 1, 2))
    # L4: [128, 10, 128]
    wq = _qint(inputs["w4"])  # [128, 128, 3, 3]
    w4p = np.zeros((128, 10, 128), np.float32)
    for ti, (dy, dx) in enumerate(TAPS):
        w4p[:, ti, :] = wq[:, :, dy, dx].T
    wc["w4sb"] = np.ascontiguousarray(w4p.astype(f8))
    wc["sb4"] = np.ascontiguousarray(
        _scale_bias(inputs["g4"], inputs["b4"], 49.0).reshape(128, 1, 2))
    # L5: [128, 10, 256]
    wq = _qint(inputs["w5"])  # [256, 128, 3, 3]
    w5p = np.zeros((128, 10, 256), np.float32)
    for ti, (dy, dx) in enumerate(TAPS):
        w5p[:, ti, :] = wq[:, :, dy, dx].T
    wc["w5sb"] = np.ascontiguousarray(w5p.astype(f8))
    wc["sb5"] = np.ascontiguousarray(
        _scale_bias(inputs["g5"], inputs["b5"], 49.0).reshape(2, 128, 2)
        .transpose(1, 0, 2))
    # L6: [128, 2, 10, 256] (cig-major)
    wq = _qint(inputs["w6"])  # [256, 256, 3, 3]
    w6p = np.zeros((128, 2, 10, 256), np.float32)
    for cig in range(2):
        for ti, (dy, dx) in enumerate(TAPS):
            w6p[:, cig, ti, :] = wq[:, cig * 128:(cig + 1) * 128, dy, dx].T
    wc["w6sb"] = np.ascontiguousarray(w6p.astype(f8))
    wc["sb6"] = np.ascontiguousarray(
        _scale_bias(inputs["g6"], inputs["b6"], 49.0).reshape(2, 128, 2)
        .transpose(1, 0, 2))
    # FC1 [512, 4096]: k=(c,y,x), c=cig*128+p  -> [128, (cig,16,512)]
    t = _qint(inputs["wf1"]).T.reshape(2, 128, 16, 512)
    wc["wf1sb"] = np.ascontiguousarray(
        np.transpose(t, (1, 0, 2, 3)).reshape(128, 2 * 16 * 512).astype(f8))
    wc["sbf1"] = np.ascontiguousarray(_scale_bias(
        inputs["gf1"], inputs["bf1"], 49.0).reshape(4, 128, 2).transpose(
        1, 0, 2))
    # FC2 [10, 512] -> [128, (4,10)]
    t = _qint(inputs["wf2"]).T.reshape(4, 128, 10)
    wc["wf2sb"] = np.ascontiguousarray(
        np.transpose(t, (1, 0, 2)).reshape(128, 40).astype(f8))
    wc["sbf2"] = _scale_bias(inputs["gf2"], inputs["bf2"], 49.0)

    x = np.asarray(inputs["x"], np.float32)
    maps = []
    for c in range(N_CORES):
        m = dict(wc)
        m["xcol"] = _im2col_bf16x3(x[c * B:(c + 1) * B])
        maps.append(m)
    return maps


# ----------------------------------------------------------------------------
# Bass program
# ----------------------------------------------------------------------------

def build_nc():
    import concourse.bacc as bacc
    import concourse.mybir as mybir
    import concourse.tile as tile

    dt = mybir.dt
    AF = mybir.ActivationFunctionType
    OP = mybir.AluOpType
    FP8 = dt.float8e4
    DR = mybir.MatmulPerfMode.DoubleRow

    nc = bacc.Bacc("TRN2", target_bir_lowering=False, debug=False)

    xcold = nc.dram_tensor("xcol", [81, 32, 32, B], dt.bfloat16,
                           kind="ExternalInput")
    w1d = nc.dram_tensor("w1sb", [81, 64], dt.bfloat16, kind="ExternalInput")
    w2d = nc.dram_tensor("w2sb", [128, 10, 128], FP8, kind="ExternalInput")
    w3d = nc.dram_tensor("w3sb", [128, 6, 128], FP8, kind="ExternalInput")
    w4d = nc.dram_tensor("w4sb", [128, 10, 128], FP8, kind="ExternalInput")
    w5d = nc.dram_tensor("w5sb", [128, 10, 256], FP8, kind="ExternalInput")
    w6d = nc.dram_tensor("w6sb", [128, 2, 10, 256], FP8, kind="ExternalInput")
    sbd = {}
    sbshape = {1: [128, 1, 2], 2: [128, 1, 2], 3: [128, 1, 2],
               4: [128, 1, 2], 5: [128, 2, 2], 6: [128, 2, 2]}
    for i in range(1, 7):
        sbd[i] = nc.dram_tensor(f"sb{i}", sbshape[i], dt.float32,
                                kind="ExternalInput")
    wf1d = nc.dram_tensor("wf1sb", [128, 2 * 16 * 512], FP8,
                          kind="ExternalInput")
    sbf1d = nc.dram_tensor("sbf1", [128, 4, 2], dt.float32,
                           kind="ExternalInput")
    wf2d = nc.dram_tensor("wf2sb", [128, 40], FP8, kind="ExternalInput")
    sbf2d = nc.dram_tensor("sbf2", [10, 2], dt.float32, kind="ExternalInput")
    outd = nc.dram_tensor("out", [B, 10], dt.float32, kind="ExternalOutput")

    with tile.TileContext(nc) as tc:
        # ------------- persistent weights ---------------------------------
        wp_cm = tc.tile_pool(name="weights", bufs=1)
        wp = wp_cm.__enter__()
        w1 = wp.tile([81, 64], dt.bfloat16, tag="w1")
        nc.sync.dma_start(w1[:], w1d[:])
        wsb = {}
        for i, shape in ((2, [128, 10, 128]), (3, [128, 6, 128]),
                         (4, [128, 10, 128]), (5, [128, 10, 256]),
                         (6, [128, 2, 10, 256])):
            t = wp.tile(shape, FP8, tag=f"w{i}")
            wsb[i] = t
        # w2 is needed right after L1; the rest stream in behind the xcol
        # chunks (their dma_starts are issued after the L1 loop below)
        nc.scalar.dma_start(wsb[2][:], w2d[:])
        sbt = {}
        sbt0 = {}
        for i in range(1, 7):
            t0 = wp.tile(sbshape[i], dt.float32, tag=f"s{i}d")
            sbt0[i] = t0
            t = wp.tile(sbshape[i], dt.float32, tag=f"s{i}")
            sbt[i] = t
        nc.sync.dma_start(sbt0[1][:], sbd[1][:])
        nc.scalar.copy(sbt[1][:], sbt0[1][:])
        wf2 = wp.tile([128, 40], FP8, tag="wf2")
        sbf2t = wp.tile([10, 2], dt.float32, tag="sf2d")
        sbf2 = wp.tile([10, 2], dt.float32, tag="sf2")

        def zero_border(A, Hp, eng=None):
            e = eng or nc.gpsimd
            e.memset(A[:, 0, :, :], 0.0)
            e.memset(A[:, Hp - 1, :, :], 0.0)
            e.memset(A[:, 1:Hp - 1, 0, :], 0.0)
            e.memset(A[:, 1:Hp - 1, Hp - 1, :], 0.0)

        def with_pair(ap, delta):
            APc = type(ap)
            pairs = list(ap.ap)
            return APc(ap.tensor, ap.offset,
                       [pairs[0], [delta, 2]] + list(pairs[1:]))

        def dr_group(ps_ap, wt, co_sl, base_fn, deltas, npairs, extra=None):
            """Accumulate npairs DR matmuls (+ optional extra groups)."""
            for p in range(npairs):
                nc.tensor.matmul(ps_ap, wt[:, 2 * p:2 * p + 2, co_sl],
                                 with_pair(base_fn(p), deltas[p]),
                                 start=(p == 0), stop=(extra is None
                                                       and p == npairs - 1),
                                 perf_mode=DR)
            if extra is not None:
                wt2, base_fn2, deltas2 = extra
                for p in range(npairs):
                    nc.tensor.matmul(ps_ap, wt2[:, 2 * p:2 * p + 2, co_sl],
                                     with_pair(base_fn2(p), deltas2[p]),
                                     start=False, stop=(p == npairs - 1),
                                     perf_mode=DR)

        # ------------- Layer 1: K=81 im2col conv, x-halves in PSUM --------
        # PSUM partitions 0-63 = left 16 output cols, 64-127 = right 16.
        # The quantized row then writes A2's lower (channels, x) AND upper
        # (x+16 view) halves in ONE vector op; only two boundary columns
        # (upper col0 = real x15, lower col17 = real x16) need patch DMAs.
        pa2_cm = tc.tile_pool(name="A2", bufs=1)
        pa2 = pa2_cm.__enter__()
        A2 = pa2.tile([128, 34, 18, B], FP8, tag="A2")
        nc.vector.memset(A2[:, 0, :, :], 0.0)
        nc.vector.memset(A2[:, 33, :, :], 0.0)
        nc.vector.memset(A2[:, 1:33, 0, :], 0.0)
        nc.vector.memset(A2[:, 1:33, 17, :], 0.0)
        pxc_cm = tc.tile_pool(name="l1xc", bufs=1)
        pxc = pxc_cm.__enter__()
        xc = pxc.tile([81, 32, 32, B], dt.bfloat16, tag="xc")
        dma_engs = (nc.sync, nc.scalar, nc.gpsimd)
        for q in range(16):  # all xcol transfers up-front, fully pipelined
            dma_engs[q % 3].dma_start(xc[:, 2 * q:2 * q + 2, :, :],
                                      xcold[:, 2 * q:2 * q + 2, :, :])
        with (tc.tile_pool(name="l1ps", bufs=2, space="PSUM") as pps,
              tc.tile_pool(name="l1z", bufs=2) as pz):
            for q in range(16):  # y-pairs
                ps = pps.tile([128, 4, 512], dt.float32, tag="ps")
                for yy in range(2):
                    y = 2 * q + yy
                    for g in range(2):
                        nc.tensor.matmul(ps[0:64, 2 * yy + g, :], w1[:],
                                         xc[:, y, g * 8:g * 8 + 8, :],
                                         start=True, stop=True)
                        nc.tensor.matmul(ps[64:128, 2 * yy + g, :], w1[:],
                                         xc[:, y, 16 + g * 8:
                                             16 + g * 8 + 8, :],
                                         start=True, stop=True)
                z7 = pz.tile([128, 4, 512], dt.float32, tag="z")
                nc.scalar.activation(z7[:], ps[:], AF.Relu,
                                     bias=sbt[1][:, 0, 1:2],
                                     scale=sbt[1][:, 0, 0:1])
                zf = z7[:].rearrange("p a b -> p (a b)")
                nc.vector.tensor_scalar(zf, zf, MAGIC, MAGIC, OP.add, OP.max)
                for yy in range(2):
                    nc.vector.tensor_scalar(
                        A2[:, 1 + 2 * q + yy, 1:17, :].rearrange(
                            "p x b -> p (x b)"),
                        z7[:, 2 * yy:2 * yy + 2, :].rearrange(
                            "p g b -> p (g b)"),
                        MAGIC, 7.0, OP.subtract, OP.min)
        # boundary patch columns (after all xcol chunks: keep queues clean)
        for k in range(4):
            r0 = 1 + 8 * k
            nc.gpsimd.dma_start(A2[64:128, r0:r0 + 8, 0:1, :],
                                A2[0:64, r0:r0 + 8, 16:17, :])
            nc.gpsimd.dma_start(A2[0:64, r0:r0 + 8, 17:18, :],
                                A2[64:128, r0:r0 + 8, 1:2, :])
        # stream the remaining weights behind the L1 loads
        nc.scalar.dma_start(wsb[3][:], w3d[:])
        nc.scalar.dma_start(wsb[4][:], w4d[:])
        nc.sync.dma_start(wsb[5][:], w5d[:])
        nc.sync.dma_start(wsb[6][:], w6d[:])
        for i in range(2, 7):
            nc.sync.dma_start(sbt0[i][:], sbd[i][:])
            nc.scalar.copy(sbt[i][:], sbt0[i][:])
        nc.sync.dma_start(wf2[:], wf2d[:])
        nc.sync.dma_start(sbf2t[:], sbf2d[:])
        nc.scalar.copy(sbf2[:], sbf2t[:])
        pxc_cm.__exit__(None, None, None)  # free the xcol staging space
        fcw_cm = tc.tile_pool(name="fcw", bufs=1)
        fcw = fcw_cm.__enter__()
        wf1 = fcw.tile([128, 2 * 16 * 512], FP8, tag="wf1")
        sbf1t = fcw.tile([128, 4, 2], dt.float32, tag="sf1d")
        sbf1 = fcw.tile([128, 4, 2], dt.float32, tag="sf1")
        nc.scalar.dma_start(sbf1t[:], sbf1d[:])
        nc.scalar.copy(sbf1[:], sbf1t[:])

        # ------------- Layer 2 (64ch 32x32, halves, pool -> 16) -----------
        pa3_cm = tc.tile_pool(name="A3", bufs=1)
        pa3 = pa3_cm.__enter__()
        A3 = pa3.tile([128, 18, 18, B], FP8, tag="A3")
        zero_border(A3, 18)
        d2 = _pair_deltas(18)
        with (tc.tile_pool(name="c2ps", bufs=3, space="PSUM") as pps,
              tc.tile_pool(name="c2z", bufs=2) as pz,
              tc.tile_pool(name="c2t", bufs=2) as pt):
            w2, sb2 = wsb[2], sbt[2]
            for yo in range(16):
                z7 = pz.tile([128, 2, 2, 8, B], dt.float32, tag="z")
                for yy in range(2):
                    y = 2 * yo + yy
                    ps = pps.tile([128, 2, 512], dt.float32, tag="ps")
                    for xh in range(2):
                        x0 = 8 * xh

                        def mkbase(p, _y=y, _x0=x0):
                            dy, dx = TAPS[2 * p] if p < 4 else TAPS[8]
                            return A2[:, _y + dy, _x0 + dx:_x0 + dx + 8, :]
                        dr_group(ps[:, xh, :], w2, slice(0, 128), mkbase,
                                 d2, 5)
                    nc.scalar.activation(
                        z7[:, yy, :, :, :].rearrange("p a x b -> p (a x b)")
                        .rearrange("p (a b) -> p a b", b=512),
                        ps[:], AF.Relu, bias=sb2[:, 0, 1:2],
                        scale=sb2[:, 0, 0:1])
                zx = pt.tile([128, 2, 2, 4, B], dt.float32, tag="zx")
                for yy in range(2):
                    nc.vector.tensor_tensor(
                        zx[:, yy, :, :, :], z7[:, yy, :, 0::2, :],
                        z7[:, yy, :, 1::2, :], OP.max)
                zp = pt.tile([128, 2, 4, B], dt.float32, tag="zp")
                nc.vector.tensor_tensor(zp[:], zx[:, 0, :, :, :],
                                        zx[:, 1, :, :, :], OP.max)
                zpf = zp[:].rearrange("p a x b -> p (a x b)")
                nc.vector.tensor_scalar(zpf, zpf, MAGIC, MAGIC,
                                        OP.add, OP.max)
                a3t = pt.tile([128, 2, 4, B], FP8, tag="a3t")
                nc.vector.tensor_scalar(
                    a3t[:].rearrange("p a x b -> p (a x b)"), zpf, MAGIC, 7.0,
                    OP.subtract, OP.min)
                nc.sync.dma_start(A3[0:64, 1 + yo, 1:9, :],
                                  a3t[0:64].rearrange("p a x b -> p (a x) b"))
                nc.sync.dma_start(A3[0:64, 1 + yo, 9:17, :],
                                  a3t[64:128].rearrange(
                                      "p a x b -> p (a x) b"))
                # x+1 dup copy for L3 dx-packing, 4 chunks
                if yo in (2, 7, 12, 15):
                    r0, r1 = {2: (0, 4), 7: (4, 9), 12: (9, 14),
                              15: (14, 18)}[yo]
                    nc.gpsimd.dma_start(A3[64:128, r0:r1, 0:17, :],
                                        A3[0:64, r0:r1, 1:18, :])

        nc.gpsimd.dma_start(wf1[:], wf1d[:])

        # ------------- Layer 3 (64 -> 128, 16x16, dx-packed) --------------
        pa4_cm = tc.tile_pool(name="A4", bufs=1)
        pa4 = pa4_cm.__enter__()
        A4 = pa4.tile([128, 18, 18, B], FP8, tag="A4")
        zero_border(A4, 18)
        d3 = [2 * B, 2 * B, 2 * B]
        with (tc.tile_pool(name="c3ps", bufs=3, space="PSUM") as pps,
              tc.tile_pool(name="c3z", bufs=2) as pz):
            w3, sb3 = wsb[3], sbt[3]
            for y in range(16):
                ps = pps.tile([128, 2, 512], dt.float32, tag="ps")
                for xh in range(2):
                    x0 = 8 * xh
                    for dy in range(3):
                        base = A3[:, y + dy, x0:x0 + 8, :]
                        nc.tensor.matmul(
                            ps[:, xh, :], w3[:, 2 * dy:2 * dy + 2, :],
                            with_pair(base, 2 * B), start=(dy == 0),
                            stop=(dy == 2), perf_mode=DR)
                z7 = pz.tile([128, 2, 8, B], dt.float32, tag="z")
                nc.scalar.activation(
                    z7[:].rearrange("p a x b -> p (a x b)")
                    .rearrange("p (a b) -> p a b", b=512),
                    ps[:], AF.Relu, bias=sb3[:, 0, 1:2], scale=sb3[:, 0, 0:1])
                zf = z7[:].rearrange("p a x b -> p (a x b)")
                nc.vector.tensor_scalar(zf, zf, MAGIC, MAGIC, OP.add, OP.max)
                nc.vector.tensor_scalar(
                    A4[:, 1 + y, 1:17, :].rearrange("p x b -> p (x b)"),
                    zf, MAGIC, 7.0, OP.subtract, OP.min)

        # ------------- Layer 4 (128 -> 128, 16x16, pool -> 8) -------------
        pa5_cm = tc.tile_pool(name="A5", bufs=1)
        pa5 = pa5_cm.__enter__()
        A5 = pa5.tile([128, 10, 10, B], FP8, tag="A5")
        zero_border(A5, 10)
        d4 = _pair_deltas(18)
        with (tc.tile_pool(name="c4ps", bufs=3, space="PSUM") as pps,
              tc.tile_pool(name="c4z", bufs=2) as pz,
              tc.tile_pool(name="c4t", bufs=2) as pt):
            w4, sb4 = wsb[4], sbt[4]
            for yo in range(8):
                z7 = pz.tile([128, 2, 2, 8, B], dt.float32, tag="z")
                for yy in range(2):
                    y = 2 * yo + yy
                    ps = pps.tile([128, 2, 512], dt.float32, tag="ps")
                    for xh in range(2):
                        x0 = 8 * xh

                        def mkbase(p, _y=y, _x0=x0):
                            dy, dx = TAPS[2 * p] if p < 4 else TAPS[8]
                            return A4[:, _y + dy, _x0 + dx:_x0 + dx + 8, :]
                        dr_group(ps[:, xh, :], w4, slice(0, 128), mkbase,
                                 d4, 5)
                    nc.scalar.activation(
                        z7[:, yy, :, :, :].rearrange("p a x b -> p (a x b)")
                        .rearrange("p (a b) -> p a b", b=512),
                        ps[:], AF.Relu, bias=sb4[:, 0, 1:2],
                        scale=sb4[:, 0, 0:1])
                zx = pt.tile([128, 2, 2, 4, B], dt.float32, tag="zx")
                for yy in range(2):
                    nc.vector.tensor_tensor(
                        zx[:, yy, :, :, :], z7[:, yy, :, 0::2, :],
                        z7[:, yy, :, 1::2, :], OP.max)
                zp = pt.tile([128, 2, 4, B], dt.float32, tag="zp")
                nc.vector.tensor_tensor(zp[:], zx[:, 0, :, :, :],
                                        zx[:, 1, :, :, :], OP.max)
                zpf = zp[:].rearrange("p a x b -> p (a x b)")
                nc.vector.tensor_scalar(zpf, zpf, MAGIC, MAGIC,
                                        OP.add, OP.max)
                nc.vector.tensor_scalar(
                    A5[:, 1 + yo, 1:9, :].rearrange("p x b -> p (x b)"),
                    zpf, MAGIC, 7.0, OP.subtract, OP.min)

        # ------------- Layer 5 (128 -> 256, 8x8) --------------------------
        pa6_cm = tc.tile_pool(name="A6", bufs=1)
        pa6 = pa6_cm.__enter__()
        A6 = pa6.tile([128, 2, 10, 10, B], FP8, tag="A6")
        nc.gpsimd.memset(A6[:, :, 0, :, :], 0.0)
        nc.gpsimd.memset(A6[:, :, 9, :, :], 0.0)
        nc.gpsimd.memset(A6[:, :, 1:9, 0, :], 0.0)
        nc.gpsimd.memset(A6[:, :, 1:9, 9, :], 0.0)
        d5 = _pair_deltas(10)
        with (tc.tile_pool(name="c5ps", bufs=3, space="PSUM") as pps,
              tc.tile_pool(name="c5z", bufs=2) as pz):
            w5, sb5 = wsb[5], sbt[5]
            for y in range(8):
                ps = pps.tile([128, 2, 512], dt.float32, tag="ps")
                for ct in range(2):
                    def mkbase(p, _y=y):
                        dy, dx = TAPS[2 * p] if p < 4 else TAPS[8]
                        return A5[:, _y + dy, dx:dx + 8, :]
                    dr_group(ps[:, ct, :], w5,
                             slice(ct * 128, ct * 128 + 128), mkbase, d5, 5)
                z7 = pz.tile([128, 2, 8, B], dt.float32, tag="z")
                for ct in range(2):
                    nc.scalar.activation(
                        z7[:, ct, :, :].rearrange("p x b -> p (x b)"),
                        ps[:, ct, :],
                        AF.Relu, bias=sb5[:, ct, 1:2], scale=sb5[:, ct, 0:1])
                zf = z7[:].rearrange("p c x b -> p (c x b)")
                nc.vector.tensor_scalar(zf, zf, MAGIC, MAGIC, OP.add, OP.max)
                for ct in range(2):
                    nc.vector.tensor_scalar(
                        A6[:, ct, 1 + y, 1:9, :].rearrange(
                            "p x b -> p (x b)"),
                        z7[:, ct, :, :].rearrange("p x b -> p (x b)"),
                        MAGIC, 7.0, OP.subtract, OP.min)

        # ------------- Layer 6 (256 -> 256, 8x8, pool -> 4) ---------------
        pa7_cm = tc.tile_pool(name="A7", bufs=1)
        pa7 = pa7_cm.__enter__()
        A7 = pa7.tile([128, 2, 4, 4, B], FP8, tag="A7")  # unpadded, feeds FC
        d6 = _pair_deltas(10)
        with (tc.tile_pool(name="c6ps", bufs=3, space="PSUM") as pps,
              tc.tile_pool(name="c6z", bufs=2) as pz,
              tc.tile_pool(name="c6t", bufs=2) as pt):
            w6, sb6 = wsb[6], sbt[6]
            for yo in range(4):
                z7 = pz.tile([128, 2, 2, 8, B], dt.float32, tag="z")
                for yy in range(2):
                    y = 2 * yo + yy
                    ps = pps.tile([128, 2, 512], dt.float32, tag="ps")
                    for ct in range(2):
                        def mkb0(p, _y=y):
                            dy, dx = TAPS[2 * p] if p < 4 else TAPS[8]
                            return A6[:, 0, _y + dy, dx:dx + 8, :]

                        def mkb1(p, _y=y):
                            dy, dx = TAPS[2 * p] if p < 4 else TAPS[8]
                            return A6[:, 1, _y + dy, dx:dx + 8, :]
                        dr_group(ps[:, ct, :], w6[:, 0],
                                 slice(ct * 128, ct * 128 + 128), mkb0, d6, 5,
                                 extra=(w6[:, 1], mkb1, d6))
                    for ct in range(2):
                        nc.scalar.activation(
                            z7[:, yy, ct, :, :].rearrange(
                                "p x b -> p (x b)"),
                            ps[:, ct, :],
                            AF.Relu, bias=sb6[:, ct, 1:2],
                            scale=sb6[:, ct, 0:1])
                zx = pt.tile([128, 2, 2, 4, B], dt.float32, tag="zx")
                for yy in range(2):
                    nc.vector.tensor_tensor(
                        zx[:, yy, :, :, :], z7[:, yy, :, 0::2, :],
                        z7[:, yy, :, 1::2, :], OP.max)
                zp = pt.tile([128, 2, 4, B], dt.float32, tag="zp")
                nc.vector.tensor_tensor(zp[:], zx[:, 0, :, :, :],
                                        zx[:, 1, :, :, :], OP.max)
                zpf = zp[:].rearrange("p c x b -> p (c x b)")
                nc.vector.tensor_scalar(zpf, zpf, MAGIC, MAGIC,
                                        OP.add, OP.max)
                for ct in range(2):
                    nc.vector.tensor_scalar(
                        A7[:, ct, yo, :, :].rearrange("p x b -> p (x b)"),
                        zp[:, ct, :, :].rearrange("p x b -> p (x b)"),
                        MAGIC, 7.0, OP.subtract, OP.min)

        # ------------- FC1 (4096 -> 512) ----------------------------------
        pa8_cm = tc.tile_pool(name="A8", bufs=1)
        pa8 = pa8_cm.__enter__()
        A8 = pa8.tile([128, 4, B], FP8, tag="A8")
        with (tc.tile_pool(name="f1ps", bufs=4, space="PSUM") as pps,
              tc.tile_pool(name="f1t", bufs=4) as pt):
            for ct in range(4):
                ps = pps.tile([128, B], dt.float32, tag="ps")
                k = 0
                for cig in range(2):
                    for px in range(16):
                        wo = (cig * 16 + px) * 512 + ct * 128
                        nc.tensor.matmul(ps[:], wf1[:, wo:wo + 128],
                                         A7[:, cig, px // 4, px % 4, :],
                                         start=(k == 0), stop=(k == 31))
                        k += 1
                z7 = pt.tile([128, B], dt.float32, tag="z")
                nc.scalar.activation(z7[:], ps[:], AF.Relu,
                                     bias=sbf1[:, ct, 1:2],
                                     scale=sbf1[:, ct, 0:1])
                nc.vector.tensor_scalar(z7[:], z7[:], MAGIC, MAGIC,
                                        OP.add, OP.max)
                nc.vector.tensor_scalar(A8[:, ct, :], z7[:], MAGIC, 7.0,
                                        OP.subtract, OP.min)

        # ------------- FC2 (512 -> 10), signed output ---------------------
        with (tc.tile_pool(name="f2ps", bufs=1, space="PSUM") as pps,
              tc.tile_pool(name="f2t", bufs=1) as pt):
            ps = pps.tile([10, B], dt.float32, tag="ps")
            for kt in range(4):
                nc.tensor.matmul(ps[:], wf2[:, kt * 10:(kt + 1) * 10],
                                 A8[:, kt, :], start=(kt == 0), stop=(kt == 3))
            z7 = pt.tile([10, B], dt.float32, tag="z")
            nc.vector.tensor_scalar(z7[:], ps[:], sbf2[:, 0:1], sbf2[:, 1:2],
                                    OP.mult, OP.add)
            r = pt.tile([10, B], dt.float32, tag="r")
            nc.vector.tensor_scalar(r[:], z7[:], MAGIC, MAGIC,
                                    OP.add, OP.subtract)  # pure RNE
            r2 = pt.tile([10, B], dt.float32, tag="r2")
            nc.vector.tensor_scalar(r2[:], r[:], -7.0, 7.0, OP.max, OP.min)
            fin = pt.tile([10, B], dt.float32, tag="fin")
            nc.vector.tensor_scalar(fin[:], r2[:], 1.0 / 7.0,
                                    None, OP.mult)
            nc.sync.dma_start(outd[:].rearrange("b c -> c b"), fin[:])
        for cm in (pa8_cm, pa7_cm, pa6_cm, pa5_cm, pa4_cm, pa3_cm, fcw_cm,
                   pa2_cm):
            cm.__exit__(None, None, None)
        wp_cm.__exit__(None, None, None)

    nc.compile()
    return nc


# ----------------------------------------------------------------------------
# Entry point
# ----------------------------------------------------------------------------

_NC_CACHE = {}
LAST_RESULTS = None  # BassKernelResults of the most recent run (for test.py)


def kernel(**inputs):
    global LAST_RESULTS
    from concourse.bass_utils import run_bass_kernel_spmd
    if "nc" not in _NC_CACHE:
        _NC_CACHE["nc"] = build_nc()
    nc = _NC_CACHE["nc"]
    in_maps = host_pack(inputs)
    res = run_bass_kernel_spmd(nc, in_maps, list(range(N_CORES)))
    LAST_RESULTS = res
    outs = [res.results[c]["out"] for c in range(N_CORES)]
    return np.concatenate(outs, axis=0).astype(np.float32)


# revision 32
# speedup vs baseline: 1.1967x; 1.1967x over previous
"""Trainium2 Bass kernel for nn_IntegerCifar10Net (quantized VGG-ish CNN).

Data parallel over 8 NeuronCores, B=64 images/core.

v2: DoubleRow fp8 matmuls contract two conv taps per moving stream
(9 taps -> 5 DR matmuls), all matmuls full 128-column, activations in
batch-major padded SBUF layout [ci, y, x, b] fp8 integers {0..7} (=7*v).

Layer schemes:
  L1 : exact 3-plane bf16 im2col (K=81), co=64; even/odd output rows go to
       PSUM partitions 0-63 / 64-127 via column-group alternation so the
       scale/round pipeline runs on all 128 lanes.
  L2 : "halves" trick - PSUM partitions 0-63 = left 16 output cols,
       64-127 = right 16 (upper SBUF partitions hold an x+16 shifted copy);
       block-diagonal [W 0; 0 W] weights, 5 DR matmuls per bank.
  L3 : upper partitions hold x+1 shifted copy (dx-packing): 6 virtual taps
       -> 3 DR matmuls per bank, co=128.
  L4-L6: plain 9 (or 2x9) taps -> 5 (10) DR matmuls per bank.
  FC : as before (N=64 matmuls are dispatch-floor bound and cheap).

Quant chain per bank: ScalarE activation (relu, per-channel scale/bias),
VectorE max-pool (free-dim pairs), RNE round via +/-1.5*2^23 magic, clamp,
cast to fp8. Shift copies (x+16 / x+1) are chunked SBUF->SBUF DMAs
interleaved with compute rows so they stay off the critical path.
"""

import sys
import numpy as np

sys.path.insert(0, "/opt/trn_rl_repo")

import ml_dtypes

N_CORES = 8
B = 64  # images per core
MAGIC = 12582912.0  # 1.5 * 2^23 : RNE rounding magic for |v| < 2^22
L1_ALT = True  # alternate PSUM column groups for even/odd L1 rows

# tap pair schedule for 3x3 convs: pairs of taps t=(dy,dx) row-major,
# 10th tap is zero-weight padding with moving delta -B (always in bounds)
TAPS = [(dy, dx) for dy in range(3) for dx in range(3)]


def _pair_deltas(W):
    """Moving-AP element deltas between the two taps of each DR pair."""
    ds = []
    for p in range(4):
        (dya, dxa), (dyb, dxb) = TAPS[2 * p], TAPS[2 * p + 1]
        ds.append(((dyb - dya) * W + (dxb - dxa)) * B)
    ds.append(-B)  # pad pair: (t8, zero-weight tap at x-1)
    return ds


# ----------------------------------------------------------------------------
# Host-side packing
# ----------------------------------------------------------------------------

def _qint(w):
    """round(clip(w,-1,1)*7) as float32 integers, matching jax fp32 chain."""
    w = np.asarray(w, np.float32)
    return np.round(np.clip(w, -1.0, 1.0) * np.float32(7.0)).astype(np.float32)


def _scale_bias(g, b, denom):
    # z7 = conv_int * (7*g/denom) + 7*b, constants in f64 then rounded to f32
    s = (7.0 * np.asarray(g, np.float64) / denom).astype(np.float32)
    bt = (7.0 * np.asarray(b, np.float64)).astype(np.float32)
    return np.ascontiguousarray(np.stack([s, bt], axis=1))  # [co, 2] f32


def _im2col_bf16x3(x):
    """x [B,3,32,32] f32 -> [81, 32, 32, B] bf16 (hi/mid/lo x 27 rows)."""
    Bc = x.shape[0]
    xp = np.zeros((Bc, 3, 34, 34), np.float32)
    xp[:, :, 1:33, 1:33] = x
    planes = np.empty((27, 32, 32, Bc), np.float32)
    k = 0
    for ci in range(3):
        for dy in range(3):
            for dx in range(3):
                planes[k] = np.transpose(xp[:, ci, dy:dy + 32, dx:dx + 32],
                                         (1, 2, 0))
                k += 1
    hi = planes.astype(ml_dtypes.bfloat16)
    r1 = planes - hi.astype(np.float32)
    mid = r1.astype(ml_dtypes.bfloat16)
    lo = (r1 - mid.astype(np.float32)).astype(ml_dtypes.bfloat16)
    return np.ascontiguousarray(np.concatenate([hi, mid, lo], axis=0))


def host_pack(inputs):
    """Build the per-core DRAM input dicts (weights replicated)."""
    f8 = ml_dtypes.float8_e4m3
    wc = {}
    # L1 weights: [64,3,3,3] -> lhsT [27,64], tripled to [81,64] bf16
    t = np.transpose(_qint(inputs["w1"]), (1, 2, 3, 0)).reshape(27, 64)
    wc["w1sb"] = np.ascontiguousarray(
        np.concatenate([t, t, t], axis=0).astype(ml_dtypes.bfloat16))
    sb = _scale_bias(inputs["g1"], inputs["b1"], 7.0)  # [64, 2]
    wc["sb1"] = np.ascontiguousarray(
        np.concatenate([sb, sb], axis=0).reshape(128, 1, 2))
    # L2: halves block-diagonal [128, 10, 128]
    wq = _qint(inputs["w2"])  # [64co, 64ci, 3, 3]
    w2p = np.zeros((128, 10, 128), np.float32)
    for ti, (dy, dx) in enumerate(TAPS):
        blk = wq[:, :, dy, dx].T  # [ci, co]
        w2p[0:64, ti, 0:64] = blk
        w2p[64:128, ti, 64:128] = blk
    wc["w2sb"] = np.ascontiguousarray(w2p.astype(f8))
    sb = _scale_bias(inputs["g2"], inputs["b2"], 49.0)
    wc["sb2"] = np.ascontiguousarray(
        np.concatenate([sb, sb], axis=0).reshape(128, 1, 2))
    # L3: dx-packed virtual taps [128, 6, 128]
    wq = _qint(inputs["w3"])  # [128co, 64ci, 3, 3]
    w3p = np.zeros((128, 6, 128), np.float32)
    for dy in range(3):
        for oi, o in enumerate((0, 2)):
            v = 2 * dy + oi
            w3p[0:64, v, :] = wq[:, :, dy, o].T
            if o == 0:
                w3p[64:128, v, :] = wq[:, :, dy, 1].T
    wc["w3sb"] = np.ascontiguousarray(w3p.astype(f8))
    wc["sb3"] = np.ascontiguousarray(
        _scale_bias(inputs["g3"], inputs["b3"], 49.0).reshape(128, 1, 2))
    # L4: [128, 10, 128]
    wq = _qint(inputs["w4"])  # [128, 128, 3, 3]
    w4p = np.zeros((128, 10, 128), np.float32)
    for ti, (dy, dx) in enumerate(TAPS):
        w4p[:, ti, :] = wq[:, :, dy, dx].T
    wc["w4sb"] = np.ascontiguousarray(w4p.astype(f8))
    wc["sb4"] = np.ascontiguousarray(
        _scale_bias(inputs["g4"], inputs["b4"], 49.0).reshape(128, 1, 2))
    # L5: [128, 10, 256]
    wq = _qint(inputs["w5"])  # [256, 128, 3, 3]
    w5p = np.zeros((128, 10, 256), np.float32)
    for ti, (dy, dx) in enumerate(TAPS):
        w5p[:, ti, :] = wq[:, :, dy, dx].T
    wc["w5sb"] = np.ascontiguousarray(w5p.astype(f8))
    wc["sb5"] = np.ascontiguousarray(
        _scale_bias(inputs["g5"], inputs["b5"], 49.0).reshape(2, 128, 2)
        .transpose(1, 0, 2))
    # L6: [128, 2, 10, 256] (cig-major)
    wq = _qint(inputs["w6"])  # [256, 256, 3, 3]
    w6p = np.zeros((128, 2, 10, 256), np.float32)
    for cig in range(2):
        for ti, (dy, dx) in enumerate(TAPS):
            w6p[:, cig, ti, :] = wq[:, cig * 128:(cig + 1) * 128, dy, dx].T
    wc["w6sb"] = np.ascontiguousarray(w6p.astype(f8))
    wc["sb6"] = np.ascontiguousarray(
        _scale_bias(inputs["g6"], inputs["b6"], 49.0).reshape(2, 128, 2)
        .transpose(1, 0, 2))
    # FC1 [512, 4096]: k=(c,y,x), c=cig*128+p  -> [128, (cig,16,512)]
    t = _qint(inputs["wf1"]).T.reshape(2, 128, 16, 512)
    wc["wf1sb"] = np.ascontiguousarray(
        np.transpose(t, (1, 0, 2, 3)).reshape(128, 2 * 16 * 512).astype(f8))
    wc["sbf1"] = np.ascontiguousarray(_scale_bias(
        inputs["gf1"], inputs["bf1"], 49.0).reshape(4, 128, 2).transpose(
        1, 0, 2))
    # FC2 [10, 512] -> [128, (4,10)]
    t = _qint(inputs["wf2"]).T.reshape(4, 128, 10)
    wc["wf2sb"] = np.ascontiguousarray(
        np.transpose(t, (1, 0, 2)).reshape(128, 40).astype(f8))
    wc["sbf2"] = _scale_bias(inputs["gf2"], inputs["bf2"], 49.0)

    x = np.asarray(inputs["x"], np.float32)
    maps = []
    for c in range(N_CORES):
        m = dict(wc)
        m["xcol"] = _im2col_bf16x3(x[c * B:(c + 1) * B])
        maps.append(m)
    return maps


# ----------------------------------------------------------------------------
# Bass program
# ----------------------------------------------------------------------------

def build_nc():
    import concourse.bacc as bacc
    import concourse.mybir as mybir
    import concourse.tile as tile

    dt = mybir.dt
    AF = mybir.ActivationFunctionType
    OP = mybir.AluOpType
    FP8 = dt.float8e4
    DR = mybir.MatmulPerfMode.DoubleRow

    nc = bacc.Bacc("TRN2", target_bir_lowering=False, debug=False)

    xcold = nc.dram_tensor("xcol", [81, 32, 32, B], dt.bfloat16,
                           kind="ExternalInput")
    w1d = nc.dram_tensor("w1sb", [81, 64], dt.bfloat16, kind="ExternalInput")
    w2d = nc.dram_tensor("w2sb", [128, 10, 128], FP8, kind="ExternalInput")
    w3d = nc.dram_tensor("w3sb", [128, 6, 128], FP8, kind="ExternalInput")
    w4d = nc.dram_tensor("w4sb", [128, 10, 128], FP8, kind="ExternalInput")
    w5d = nc.dram_tensor("w5sb", [128, 10, 256], FP8, kind="ExternalInput")
    w6d = nc.dram_tensor("w6sb", [128, 2, 10, 256], FP8, kind="ExternalInput")
    sbd = {}
    sbshape = {1: [128, 1, 2], 2: [128, 1, 2], 3: [128, 1, 2],
               4: [128, 1, 2], 5: [128, 2, 2], 6: [128, 2, 2]}
    for i in range(1, 7):
        sbd[i] = nc.dram_tensor(f"sb{i}", sbshape[i], dt.float32,
                                kind="ExternalInput")
    wf1d = nc.dram_tensor("wf1sb", [128, 2 * 16 * 512], FP8,
                          kind="ExternalInput")
    sbf1d = nc.dram_tensor("sbf1", [128, 4, 2], dt.float32,
                           kind="ExternalInput")
    wf2d = nc.dram_tensor("wf2sb", [128, 40], FP8, kind="ExternalInput")
    sbf2d = nc.dram_tensor("sbf2", [10, 2], dt.float32, kind="ExternalInput")
    outd = nc.dram_tensor("out", [B, 10], dt.float32, kind="ExternalOutput")

    with tile.TileContext(nc) as tc:
        # ------------- persistent weights ---------------------------------
        wp_cm = tc.tile_pool(name="weights", bufs=1)
        wp = wp_cm.__enter__()
        w1 = wp.tile([81, 64], dt.bfloat16, tag="w1")
        nc.sync.dma_start(w1[:], w1d[:])
        wsb = {}
        for i, shape in ((2, [128, 10, 128]), (3, [128, 6, 128]),
                         (4, [128, 10, 128]), (5, [128, 10, 256]),
                         (6, [128, 2, 10, 256])):
            t = wp.tile(shape, FP8, tag=f"w{i}")
            wsb[i] = t
        # w2 is needed right after L1; the rest stream in behind the xcol
        # chunks (their dma_starts are issued after the L1 loop below)
        nc.scalar.dma_start(wsb[2][:], w2d[:])
        sbt = {}
        sbt0 = {}
        for i in range(1, 7):
            t0 = wp.tile(sbshape[i], dt.float32, tag=f"s{i}d")
            sbt0[i] = t0
            t = wp.tile(sbshape[i], dt.float32, tag=f"s{i}")
            sbt[i] = t
        nc.sync.dma_start(sbt0[1][:], sbd[1][:])
        nc.scalar.copy(sbt[1][:], sbt0[1][:])
        wf2 = wp.tile([128, 40], FP8, tag="wf2")
        sbf2t = wp.tile([10, 2], dt.float32, tag="sf2d")
        sbf2 = wp.tile([10, 2], dt.float32, tag="sf2")

        def zero_border(A, Hp, eng=None):
            e = eng or nc.gpsimd
            e.memset(A[:, 0, :, :], 0.0)
            e.memset(A[:, Hp - 1, :, :], 0.0)
            e.memset(A[:, 1:Hp - 1, 0, :], 0.0)
            e.memset(A[:, 1:Hp - 1, Hp - 1, :], 0.0)

        def with_pair(ap, delta):
            APc = type(ap)
            pairs = list(ap.ap)
            return APc(ap.tensor, ap.offset,
                       [pairs[0], [delta, 2]] + list(pairs[1:]))

        def dr_group(ps_ap, wt, co_sl, base_fn, deltas, npairs, extra=None):
            """Accumulate npairs DR matmuls (+ optional extra groups)."""
            for p in range(npairs):
                nc.tensor.matmul(ps_ap, wt[:, 2 * p:2 * p + 2, co_sl],
                                 with_pair(base_fn(p), deltas[p]),
                                 start=(p == 0), stop=(extra is None
                                                       and p == npairs - 1),
                                 perf_mode=DR)
            if extra is not None:
                wt2, base_fn2, deltas2 = extra
                for p in range(npairs):
                    nc.tensor.matmul(ps_ap, wt2[:, 2 * p:2 * p + 2, co_sl],
                                     with_pair(base_fn2(p), deltas2[p]),
                                     start=False, stop=(p == npairs - 1),
                                     perf_mode=DR)

        # ------------- Layer 1: K=81 im2col conv, x-halves in PSUM --------
        # PSUM partitions 0-63 = left 16 output cols, 64-127 = right 16.
        # The quantized row then writes A2's lower (channels, x) AND upper
        # (x+16 view) halves in ONE vector op; only two boundary columns
        # (upper col0 = real x15, lower col17 = real x16) need patch DMAs.
        pa2_cm = tc.tile_pool(name="A2", bufs=1)
        pa2 = pa2_cm.__enter__()
        A2 = pa2.tile([128, 34, 18, B], FP8, tag="A2")
        nc.vector.memset(A2[:, 0, :, :], 0.0)
        nc.vector.memset(A2[:, 33, :, :], 0.0)
        nc.vector.memset(A2[:, 1:33, 0, :], 0.0)
        nc.vector.memset(A2[:, 1:33, 17, :], 0.0)
        pxc_cm = tc.tile_pool(name="l1mov", bufs=9)
        pxc = pxc_cm.__enter__()
        dma_engs = (nc.sync, nc.scalar, nc.gpsimd)
        with (tc.tile_pool(name="l1ps", bufs=2, space="PSUM") as pps,
              tc.tile_pool(name="l1z", bufs=2) as pz):
            for q in range(16):  # y-pairs
                mov = pxc.tile([81, 2, 32, B], dt.bfloat16, tag="mov")
                dma_engs[q % 3].dma_start(mov[:], xcold[:, 2 * q:2 * q + 2,
                                                        :, :])
                ps = pps.tile([128, 4, 512], dt.float32, tag="ps")
                for yy in range(2):
                    for g in range(2):
                        nc.tensor.matmul(ps[0:64, 2 * yy + g, :], w1[:],
                                         mov[:, yy, g * 8:g * 8 + 8, :],
                                         start=True, stop=True)
                        nc.tensor.matmul(ps[64:128, 2 * yy + g, :], w1[:],
                                         mov[:, yy, 16 + g * 8:
                                              16 + g * 8 + 8, :],
                                         start=True, stop=True)
                z7 = pz.tile([128, 4, 512], dt.float32, tag="z")
                nc.scalar.activation(z7[:], ps[:], AF.Relu,
                                     bias=sbt[1][:, 0, 1:2],
                                     scale=sbt[1][:, 0, 0:1])
                zf = z7[:].rearrange("p a b -> p (a b)")
                nc.vector.tensor_scalar(zf, zf, MAGIC, MAGIC, OP.add, OP.max)
                for yy in range(2):
                    nc.vector.tensor_scalar(
                        A2[:, 1 + 2 * q + yy, 1:17, :].rearrange(
                            "p x b -> p (x b)"),
                        z7[:, 2 * yy:2 * yy + 2, :].rearrange(
                            "p g b -> p (g b)"),
                        MAGIC, 7.0, OP.subtract, OP.min)
        # boundary patch columns (after all xcol chunks: keep queues clean)
        for k in range(4):
            r0 = 1 + 8 * k
            nc.gpsimd.dma_start(A2[64:128, r0:r0 + 8, 0:1, :],
                                A2[0:64, r0:r0 + 8, 16:17, :])
            nc.gpsimd.dma_start(A2[0:64, r0:r0 + 8, 17:18, :],
                                A2[64:128, r0:r0 + 8, 1:2, :])
        # stream the remaining weights behind the L1 loads
        nc.scalar.dma_start(wsb[3][:], w3d[:])
        nc.scalar.dma_start(wsb[4][:], w4d[:])
        nc.sync.dma_start(wsb[5][:], w5d[:])
        nc.sync.dma_start(wsb[6][:], w6d[:])
        for i in range(2, 7):
            nc.sync.dma_start(sbt0[i][:], sbd[i][:])
            nc.scalar.copy(sbt[i][:], sbt0[i][:])
        nc.sync.dma_start(wf2[:], wf2d[:])
        nc.sync.dma_start(sbf2t[:], sbf2d[:])
        nc.scalar.copy(sbf2[:], sbf2t[:])
        pxc_cm.__exit__(None, None, None)  # free the xcol staging space
        fcw_cm = tc.tile_pool(name="fcw", bufs=1)
        fcw = fcw_cm.__enter__()
        wf1 = fcw.tile([128, 2 * 16 * 512], FP8, tag="wf1")
        sbf1t = fcw.tile([128, 4, 2], dt.float32, tag="sf1d")
        sbf1 = fcw.tile([128, 4, 2], dt.float32, tag="sf1")
        nc.scalar.dma_start(sbf1t[:], sbf1d[:])
        nc.scalar.copy(sbf1[:], sbf1t[:])

        # ------------- Layer 2 (64ch 32x32, halves, pool -> 16) -----------
        pa3_cm = tc.tile_pool(name="A3", bufs=1)
        pa3 = pa3_cm.__enter__()
        A3 = pa3.tile([128, 18, 18, B], FP8, tag="A3")
        zero_border(A3, 18)
        d2 = _pair_deltas(18)
        with (tc.tile_pool(name="c2ps", bufs=3, space="PSUM") as pps,
              tc.tile_pool(name="c2z", bufs=2) as pz,
              tc.tile_pool(name="c2t", bufs=2) as pt):
            w2, sb2 = wsb[2], sbt[2]
            for yo in range(16):
                z7 = pz.tile([128, 2, 2, 8, B], dt.float32, tag="z")
                for yy in range(2):
                    y = 2 * yo + yy
                    ps = pps.tile([128, 2, 512], dt.float32, tag="ps")
                    for xh in range(2):
                        x0 = 8 * xh

                        def mkbase(p, _y=y, _x0=x0):
                            dy, dx = TAPS[2 * p] if p < 4 else TAPS[8]
                            return A2[:, _y + dy, _x0 + dx:_x0 + dx + 8, :]
                        dr_group(ps[:, xh, :], w2, slice(0, 128), mkbase,
                                 d2, 5)
                    nc.scalar.activation(
                        z7[:, yy, :, :, :].rearrange("p a x b -> p (a x b)")
                        .rearrange("p (a b) -> p a b", b=512),
                        ps[:], AF.Relu, bias=sb2[:, 0, 1:2],
                        scale=sb2[:, 0, 0:1])
                zx = pt.tile([128, 2, 2, 4, B], dt.float32, tag="zx")
                for yy in range(2):
                    nc.vector.tensor_tensor(
                        zx[:, yy, :, :, :], z7[:, yy, :, 0::2, :],
                        z7[:, yy, :, 1::2, :], OP.max)
                zp = pt.tile([128, 2, 4, B], dt.float32, tag="zp")
                nc.vector.tensor_tensor(zp[:], zx[:, 0, :, :, :],
                                        zx[:, 1, :, :, :], OP.max)
                zpf = zp[:].rearrange("p a x b -> p (a x b)")
                nc.vector.tensor_scalar(zpf, zpf, MAGIC, MAGIC,
                                        OP.add, OP.max)
                a3t = pt.tile([128, 2, 4, B], FP8, tag="a3t")
                nc.vector.tensor_scalar(
                    a3t[:].rearrange("p a x b -> p (a x b)"), zpf, MAGIC, 7.0,
                    OP.subtract, OP.min)
                nc.sync.dma_start(A3[0:64, 1 + yo, 1:9, :],
                                  a3t[0:64].rearrange("p a x b -> p (a x) b"))
                nc.sync.dma_start(A3[0:64, 1 + yo, 9:17, :],
                                  a3t[64:128].rearrange(
                                      "p a x b -> p (a x) b"))
                # x+1 dup copy for L3 dx-packing, 4 chunks
                if yo in (2, 7, 12, 15):
                    r0, r1 = {2: (0, 4), 7: (4, 9), 12: (9, 14),
                              15: (14, 18)}[yo]
                    nc.gpsimd.dma_start(A3[64:128, r0:r1, 0:17, :],
                                        A3[0:64, r0:r1, 1:18, :])

        nc.gpsimd.dma_start(wf1[:], wf1d[:])

        # ------------- Layer 3 (64 -> 128, 16x16, dx-packed) --------------
        pa4_cm = tc.tile_pool(name="A4", bufs=1)
        pa4 = pa4_cm.__enter__()
        A4 = pa4.tile([128, 18, 18, B], FP8, tag="A4")
        zero_border(A4, 18)
        d3 = [2 * B, 2 * B, 2 * B]
        with (tc.tile_pool(name="c3ps", bufs=3, space="PSUM") as pps,
              tc.tile_pool(name="c3z", bufs=2) as pz):
            w3, sb3 = wsb[3], sbt[3]
            for y in range(16):
                ps = pps.tile([128, 2, 512], dt.float32, tag="ps")
                for xh in range(2):
                    x0 = 8 * xh
                    for dy in range(3):
                        base = A3[:, y + dy, x0:x0 + 8, :]
                        nc.tensor.matmul(
                            ps[:, xh, :], w3[:, 2 * dy:2 * dy + 2, :],
                            with_pair(base, 2 * B), start=(dy == 0),
                            stop=(dy == 2), perf_mode=DR)
                z7 = pz.tile([128, 2, 8, B], dt.float32, tag="z")
                nc.scalar.activation(
                    z7[:].rearrange("p a x b -> p (a x b)")
                    .rearrange("p (a b) -> p a b", b=512),
                    ps[:], AF.Relu, bias=sb3[:, 0, 1:2], scale=sb3[:, 0, 0:1])
                zf = z7[:].rearrange("p a x b -> p (a x b)")
                nc.vector.tensor_scalar(zf, zf, MAGIC, MAGIC, OP.add, OP.max)
                nc.vector.tensor_scalar(
                    A4[:, 1 + y, 1:17, :].rearrange("p x b -> p (x b)"),
                    zf, MAGIC, 7.0, OP.subtract, OP.min)

        # ------------- Layer 4 (128 -> 128, 16x16, pool -> 8) -------------
        pa5_cm = tc.tile_pool(name="A5", bufs=1)
        pa5 = pa5_cm.__enter__()
        A5 = pa5.tile([128, 10, 10, B], FP8, tag="A5")
        zero_border(A5, 10)
        d4 = _pair_deltas(18)
        with (tc.tile_pool(name="c4ps", bufs=3, space="PSUM") as pps,
              tc.tile_pool(name="c4z", bufs=2) as pz,
              tc.tile_pool(name="c4t", bufs=2) as pt):
            w4, sb4 = wsb[4], sbt[4]
            for yo in range(8):
                z7 = pz.tile([128, 2, 2, 8, B], dt.float32, tag="z")
                for yy in range(2):
                    y = 2 * yo + yy
                    ps = pps.tile([128, 2, 512], dt.float32, tag="ps")
                    for xh in range(2):
                        x0 = 8 * xh

                        def mkbase(p, _y=y, _x0=x0):
                            dy, dx = TAPS[2 * p] if p < 4 else TAPS[8]
                            return A4[:, _y + dy, _x0 + dx:_x0 + dx + 8, :]
                        dr_group(ps[:, xh, :], w4, slice(0, 128), mkbase,
                                 d4, 5)
                    nc.scalar.activation(
                        z7[:, yy, :, :, :].rearrange("p a x b -> p (a x b)")
                        .rearrange("p (a b) -> p a b", b=512),
                        ps[:], AF.Relu, bias=sb4[:, 0, 1:2],
                        scale=sb4[:, 0, 0:1])
                zx = pt.tile([128, 2, 2, 4, B], dt.float32, tag="zx")
                for yy in range(2):
                    nc.vector.tensor_tensor(
                        zx[:, yy, :, :, :], z7[:, yy, :, 0::2, :],
                        z7[:, yy, :, 1::2, :], OP.max)
                zp = pt.tile([128, 2, 4, B], dt.float32, tag="zp")
                nc.vector.tensor_tensor(zp[:], zx[:, 0, :, :, :],
                                        zx[:, 1, :, :, :], OP.max)
                zpf = zp[:].rearrange("p a x b -> p (a x b)")
                nc.vector.tensor_scalar(zpf, zpf, MAGIC, MAGIC,
                                        OP.add, OP.max)
                nc.vector.tensor_scalar(
                    A5[:, 1 + yo, 1:9, :].rearrange("p x b -> p (x b)"),
                    zpf, MAGIC, 7.0, OP.subtract, OP.min)

        # ------------- Layer 5 (128 -> 256, 8x8) --------------------------
        pa6_cm = tc.tile_pool(name="A6", bufs=1)
        pa6 = pa6_cm.__enter__()
        A6 = pa6.tile([128, 2, 10, 10, B], FP8, tag="A6")
        nc.gpsimd.memset(A6[:, :, 0, :, :], 0.0)
        nc.gpsimd.memset(A6[:, :, 9, :, :], 0.0)
        nc.gpsimd.memset(A6[:, :, 1:9, 0, :], 0.0)
        nc.gpsimd.memset(A6[:, :, 1:9, 9, :], 0.0)
        d5 = _pair_deltas(10)
        with (tc.tile_pool(name="c5ps", bufs=3, space="PSUM") as pps,
              tc.tile_pool(name="c5z", bufs=2) as pz):
            w5, sb5 = wsb[5], sbt[5]
            for y in range(8):
                ps = pps.tile([128, 2, 512], dt.float32, tag="ps")
                for ct in range(2):
                    def mkbase(p, _y=y):
                        dy, dx = TAPS[2 * p] if p < 4 else TAPS[8]
                        return A5[:, _y + dy, dx:dx + 8, :]
                    dr_group(ps[:, ct, :], w5,
                             slice(ct * 128, ct * 128 + 128), mkbase, d5, 5)
                z7 = pz.tile([128, 2, 8, B], dt.float32, tag="z")
                for ct in range(2):
                    nc.scalar.activation(
                        z7[:, ct, :, :].rearrange("p x b -> p (x b)"),
                        ps[:, ct, :],
                        AF.Relu, bias=sb5[:, ct, 1:2], scale=sb5[:, ct, 0:1])
                zf = z7[:].rearrange("p c x b -> p (c x b)")
                nc.vector.tensor_scalar(zf, zf, MAGIC, MAGIC, OP.add, OP.max)
                for ct in range(2):
                    nc.vector.tensor_scalar(
                        A6[:, ct, 1 + y, 1:9, :].rearrange(
                            "p x b -> p (x b)"),
                        z7[:, ct, :, :].rearrange("p x b -> p (x b)"),
                        MAGIC, 7.0, OP.subtract, OP.min)

        # ------------- Layer 6 (256 -> 256, 8x8, pool -> 4) ---------------
        pa7_cm = tc.tile_pool(name="A7", bufs=1)
        pa7 = pa7_cm.__enter__()
        A7 = pa7.tile([128, 2, 4, 4, B], FP8, tag="A7")  # unpadded, feeds FC
        d6 = _pair_deltas(10)
        with (tc.tile_pool(name="c6ps", bufs=3, space="PSUM") as pps,
              tc.tile_pool(name="c6z", bufs=2) as pz,
              tc.tile_pool(name="c6t", bufs=2) as pt):
            w6, sb6 = wsb[6], sbt[6]
            for yo in range(4):
                z7 = pz.tile([128, 2, 2, 8, B], dt.float32, tag="z")
                for yy in range(2):
                    y = 2 * yo + yy
                    ps = pps.tile([128, 2, 512], dt.float32, tag="ps")
                    for ct in range(2):
                        def mkb0(p, _y=y):
                            dy, dx = TAPS[2 * p] if p < 4 else TAPS[8]
                            return A6[:, 0, _y + dy, dx:dx + 8, :]

                        def mkb1(p, _y=y):
                            dy, dx = TAPS[2 * p] if p < 4 else TAPS[8]
                            return A6[:, 1, _y + dy, dx:dx + 8, :]
                        dr_group(ps[:, ct, :], w6[:, 0],
                                 slice(ct * 128, ct * 128 + 128), mkb0, d6, 5,
                                 extra=(w6[:, 1], mkb1, d6))
                    for ct in range(2):
                        nc.scalar.activation(
                            z7[:, yy, ct, :, :].rearrange(
                                "p x b -> p (x b)"),
                            ps[:, ct, :],
                            AF.Relu, bias=sb6[:, ct, 1:2],
                            scale=sb6[:, ct, 0:1])
                zx = pt.tile([128, 2, 2, 4, B], dt.float32, tag="zx")
                for yy in range(2):
                    nc.vector.tensor_tensor(
                        zx[:, yy, :, :, :], z7[:, yy, :, 0::2, :],
                        z7[:, yy, :, 1::2, :], OP.max)
                zp = pt.tile([128, 2, 4, B], dt.float32, tag="zp")
                nc.vector.tensor_tensor(zp[:], zx[:, 0, :, :, :],
                                        zx[:, 1, :, :, :], OP.max)
                zpf = zp[:].rearrange("p c x b -> p (c x b)")
                nc.vector.tensor_scalar(zpf, zpf, MAGIC, MAGIC,
                                        OP.add, OP.max)
                for ct in range(2):
                    nc.vector.tensor_scalar(
                        A7[:, ct, yo, :, :].rearrange("p x b -> p (x b)"),
                        zp[:, ct, :, :].rearrange("p x b -> p (x b)"),
                        MAGIC, 7.0, OP.subtract, OP.min)

        # ------------- FC1 (4096 -> 512) ----------------------------------
        pa8_cm = tc.tile_pool(name="A8", bufs=1)
        pa8 = pa8_cm.__enter__()
        A8 = pa8.tile([128, 4, B], FP8, tag="A8")
        with (tc.tile_pool(name="f1ps", bufs=4, space="PSUM") as pps,
              tc.tile_pool(name="f1t", bufs=4) as pt):
            for ct in range(4):
                ps = pps.tile([128, B], dt.float32, tag="ps")
                k = 0
                for cig in range(2):
                    for px in range(16):
                        wo = (cig * 16 + px) * 512 + ct * 128
                        nc.tensor.matmul(ps[:], wf1[:, wo:wo + 128],
                                         A7[:, cig, px // 4, px % 4, :],
                                         start=(k == 0), stop=(k == 31))
                        k += 1
                z7 = pt.tile([128, B], dt.float32, tag="z")
                nc.scalar.activation(z7[:], ps[:], AF.Relu,
                                     bias=sbf1[:, ct, 1:2],
                                     scale=sbf1[:, ct, 0:1])
                nc.vector.tensor_scalar(z7[:], z7[:], MAGIC, MAGIC,
                                        OP.add, OP.max)
                nc.vector.tensor_scalar(A8[:, ct, :], z7[:], MAGIC, 7.0,
                                        OP.subtract, OP.min)

        # ------------- FC2 (512 -> 10), signed output ---------------------
        with (tc.tile_pool(name="f2ps", bufs=1, space="PSUM") as pps,
              tc.tile_pool(name="f2t", bufs=1) as pt):
            ps = pps.tile([10, B], dt.float32, tag="ps")
            for kt in range(4):
                nc.tensor.matmul(ps[:], wf2[:, kt * 10:(kt + 1) * 10],
                                 A8[:, kt, :], start=(kt == 0), stop=(kt == 3))
            z7 = pt.tile([10, B], dt.float32, tag="z")
            nc.vector.tensor_scalar(z7[:], ps[:], sbf2[:, 0:1], sbf2[:, 1:2],
                                    OP.mult, OP.add)
            r = pt.tile([10, B], dt.float32, tag="r")
            nc.vector.tensor_scalar(r[:], z7[:], MAGIC, MAGIC,
                                    OP.add, OP.subtract)  # pure RNE
            r2 = pt.tile([10, B], dt.float32, tag="r2")
            nc.vector.tensor_scalar(r2[:], r[:], -7.0, 7.0, OP.max, OP.min)
            fin = pt.tile([10, B], dt.float32, tag="fin")
            nc.vector.tensor_scalar(fin[:], r2[:], 1.0 / 7.0,
                                    None, OP.mult)
            nc.sync.dma_start(outd[:].rearrange("b c -> c b"), fin[:])
        for cm in (pa8_cm, pa7_cm, pa6_cm, pa5_cm, pa4_cm, pa3_cm, fcw_cm,
                   pa2_cm):
            cm.__exit__(None, None, None)
        wp_cm.__exit__(None, None, None)

    nc.compile()
    return nc


# ----------------------------------------------------------------------------
# Entry point
# ----------------------------------------------------------------------------

_NC_CACHE = {}
LAST_RESULTS = None  # BassKernelResults of the most recent run (for test.py)


def kernel(**inputs):
    global LAST_RESULTS
    from concourse.bass_utils import run_bass_kernel_spmd
    if "nc" not in _NC_CACHE:
        _NC_CACHE["nc"] = build_nc()
    nc = _NC_CACHE["nc"]
    in_maps = host_pack(inputs)
    res = run_bass_kernel_spmd(nc, in_maps, list(range(N_CORES)))
    LAST_RESULTS = res
    outs = [res.results[c]["out"] for c in range(N_CORES)]
    return np.concatenate(outs, axis=0).astype(np.float32)


# revision 33
# speedup vs baseline: 1.2906x; 1.0785x over previous
"""Trainium2 Bass kernel for nn_IntegerCifar10Net (quantized VGG-ish CNN).

Data parallel over 8 NeuronCores, B=64 images/core.

v2: DoubleRow fp8 matmuls contract two conv taps per moving stream
(9 taps -> 5 DR matmuls), all matmuls full 128-column, activations in
batch-major padded SBUF layout [ci, y, x, b] fp8 integers {0..7} (=7*v).

Layer schemes:
  L1 : exact 3-plane bf16 im2col (K=81), co=64; even/odd output rows go to
       PSUM partitions 0-63 / 64-127 via column-group alternation so the
       scale/round pipeline runs on all 128 lanes.
  L2 : "halves" trick - PSUM partitions 0-63 = left 16 output cols,
       64-127 = right 16 (upper SBUF partitions hold an x+16 shifted copy);
       block-diagonal [W 0; 0 W] weights, 5 DR matmuls per bank.
  L3 : upper partitions hold x+1 shifted copy (dx-packing): 6 virtual taps
       -> 3 DR matmuls per bank, co=128.
  L4-L6: plain 9 (or 2x9) taps -> 5 (10) DR matmuls per bank.
  FC : as before (N=64 matmuls are dispatch-floor bound and cheap).

Quant chain per bank: ScalarE activation (relu, per-channel scale/bias),
VectorE max-pool (free-dim pairs), RNE round via +/-1.5*2^23 magic, clamp,
cast to fp8. Shift copies (x+16 / x+1) are chunked SBUF->SBUF DMAs
interleaved with compute rows so they stay off the critical path.
"""

import sys
import numpy as np

sys.path.insert(0, "/opt/trn_rl_repo")

import ml_dtypes

N_CORES = 8
B = 64  # images per core
MAGIC = 12582912.0  # 1.5 * 2^23 : RNE rounding magic for |v| < 2^22
L1_ALT = True  # alternate PSUM column groups for even/odd L1 rows

# tap pair schedule for 3x3 convs: pairs of taps t=(dy,dx) row-major,
# 10th tap is zero-weight padding with moving delta -B (always in bounds)
TAPS = [(dy, dx) for dy in range(3) for dx in range(3)]


def _pair_deltas(W):
    """Moving-AP element deltas between the two taps of each DR pair."""
    ds = []
    for p in range(4):
        (dya, dxa), (dyb, dxb) = TAPS[2 * p], TAPS[2 * p + 1]
        ds.append(((dyb - dya) * W + (dxb - dxa)) * B)
    ds.append(-B)  # pad pair: (t8, zero-weight tap at x-1)
    return ds


# ----------------------------------------------------------------------------
# Host-side packing
# ----------------------------------------------------------------------------

def _qint(w):
    """round(clip(w,-1,1)*7) as float32 integers, matching jax fp32 chain."""
    w = np.asarray(w, np.float32)
    return np.round(np.clip(w, -1.0, 1.0) * np.float32(7.0)).astype(np.float32)


def _scale_bias(g, b, denom):
    # z7 = conv_int * (7*g/denom) + 7*b, constants in f64 then rounded to f32
    s = (7.0 * np.asarray(g, np.float64) / denom).astype(np.float32)
    bt = (7.0 * np.asarray(b, np.float64)).astype(np.float32)
    return np.ascontiguousarray(np.stack([s, bt], axis=1))  # [co, 2] f32


def _im2col_bf16x3(x):
    """x [B,3,32,32] f32 -> [81, 32, 32, B] bf16 (hi/mid/lo x 27 rows)."""
    Bc = x.shape[0]
    xp = np.zeros((Bc, 3, 34, 34), np.float32)
    xp[:, :, 1:33, 1:33] = x
    planes = np.empty((27, 32, 32, Bc), np.float32)
    k = 0
    for ci in range(3):
        for dy in range(3):
            for dx in range(3):
                planes[k] = np.transpose(xp[:, ci, dy:dy + 32, dx:dx + 32],
                                         (1, 2, 0))
                k += 1
    hi = planes.astype(ml_dtypes.bfloat16)
    r1 = planes - hi.astype(np.float32)
    mid = r1.astype(ml_dtypes.bfloat16)
    lo = (r1 - mid.astype(np.float32)).astype(ml_dtypes.bfloat16)
    return np.ascontiguousarray(np.concatenate([hi, mid, lo], axis=0))


def host_pack(inputs):
    """Build the per-core DRAM input dicts (weights replicated)."""
    f8 = ml_dtypes.float8_e4m3
    wc = {}
    # L1 weights: [64,3,3,3] -> lhsT [27,64], tripled to [81,64] bf16
    t = np.transpose(_qint(inputs["w1"]), (1, 2, 3, 0)).reshape(27, 64)
    wc["w1sb"] = np.ascontiguousarray(
        np.concatenate([t, t, t], axis=0).astype(ml_dtypes.bfloat16))
    sb = _scale_bias(inputs["g1"], inputs["b1"], 7.0)  # [64, 2]
    wc["sb1"] = np.ascontiguousarray(
        np.concatenate([sb, sb], axis=0).reshape(128, 1, 2))
    # L2: halves block-diagonal [128, 10, 128]
    wq = _qint(inputs["w2"])  # [64co, 64ci, 3, 3]
    w2p = np.zeros((128, 10, 128), np.float32)
    for ti, (dy, dx) in enumerate(TAPS):
        blk = wq[:, :, dy, dx].T  # [ci, co]
        w2p[0:64, ti, 0:64] = blk
        w2p[64:128, ti, 64:128] = blk
    wc["w2sb"] = np.ascontiguousarray(w2p.astype(f8))
    sb = _scale_bias(inputs["g2"], inputs["b2"], 49.0)
    wc["sb2"] = np.ascontiguousarray(
        np.concatenate([sb, sb], axis=0).reshape(128, 1, 2))
    # L3: dx-packed virtual taps [128, 6, 128]
    wq = _qint(inputs["w3"])  # [128co, 64ci, 3, 3]
    w3p = np.zeros((128, 6, 128), np.float32)
    for dy in range(3):
        for oi, o in enumerate((0, 2)):
            v = 2 * dy + oi
            w3p[0:64, v, :] = wq[:, :, dy, o].T
            if o == 0:
                w3p[64:128, v, :] = wq[:, :, dy, 1].T
    wc["w3sb"] = np.ascontiguousarray(w3p.astype(f8))
    wc["sb3"] = np.ascontiguousarray(
        _scale_bias(inputs["g3"], inputs["b3"], 49.0).reshape(128, 1, 2))
    # L4: [128, 10, 128]
    wq = _qint(inputs["w4"])  # [128, 128, 3, 3]
    w4p = np.zeros((128, 10, 128), np.float32)
    for ti, (dy, dx) in enumerate(TAPS):
        w4p[:, ti, :] = wq[:, :, dy, dx].T
    wc["w4sb"] = np.ascontiguousarray(w4p.astype(f8))
    wc["sb4"] = np.ascontiguousarray(
        _scale_bias(inputs["g4"], inputs["b4"], 49.0).reshape(128, 1, 2))
    # L5: [128, 10, 256]
    wq = _qint(inputs["w5"])  # [256, 128, 3, 3]
    w5p = np.zeros((128, 10, 256), np.float32)
    for ti, (dy, dx) in enumerate(TAPS):
        w5p[:, ti, :] = wq[:, :, dy, dx].T
    wc["w5sb"] = np.ascontiguousarray(w5p.astype(f8))
    wc["sb5"] = np.ascontiguousarray(
        _scale_bias(inputs["g5"], inputs["b5"], 49.0).reshape(2, 128, 2)
        .transpose(1, 0, 2))
    # L6: [128, 2, 10, 256] (cig-major)
    wq = _qint(inputs["w6"])  # [256, 256, 3, 3]
    w6p = np.zeros((128, 2, 10, 256), np.float32)
    for cig in range(2):
        for ti, (dy, dx) in enumerate(TAPS):
            w6p[:, cig, ti, :] = wq[:, cig * 128:(cig + 1) * 128, dy, dx].T
    wc["w6sb"] = np.ascontiguousarray(w6p.astype(f8))
    wc["sb6"] = np.ascontiguousarray(
        _scale_bias(inputs["g6"], inputs["b6"], 49.0).reshape(2, 128, 2)
        .transpose(1, 0, 2))
    # FC1 [512, 4096]: k=(c,y,x), c=cig*128+p  -> [128, (cig,16,512)]
    t = _qint(inputs["wf1"]).T.reshape(2, 128, 16, 512)
    wc["wf1sb"] = np.ascontiguousarray(
        np.transpose(t, (1, 0, 2, 3)).reshape(128, 2 * 16 * 512).astype(f8))
    wc["sbf1"] = np.ascontiguousarray(_scale_bias(
        inputs["gf1"], inputs["bf1"], 49.0).reshape(4, 128, 2).transpose(
        1, 0, 2))
    # FC2 [10, 512] -> [128, (4,10)]
    t = _qint(inputs["wf2"]).T.reshape(4, 128, 10)
    wc["wf2sb"] = np.ascontiguousarray(
        np.transpose(t, (1, 0, 2)).reshape(128, 40).astype(f8))
    wc["sbf2"] = _scale_bias(inputs["gf2"], inputs["bf2"], 49.0)

    x = np.asarray(inputs["x"], np.float32)
    maps = []
    for c in range(N_CORES):
        m = dict(wc)
        m["xcol"] = _im2col_bf16x3(x[c * B:(c + 1) * B])
        maps.append(m)
    return maps


# ----------------------------------------------------------------------------
# Bass program
# ----------------------------------------------------------------------------

def build_nc():
    import concourse.bacc as bacc
    import concourse.mybir as mybir
    import concourse.tile as tile

    dt = mybir.dt
    AF = mybir.ActivationFunctionType
    OP = mybir.AluOpType
    FP8 = dt.float8e4
    DR = mybir.MatmulPerfMode.DoubleRow

    nc = bacc.Bacc("TRN2", target_bir_lowering=False, debug=False)

    xcold = nc.dram_tensor("xcol", [81, 32, 32, B], dt.bfloat16,
                           kind="ExternalInput")
    w1d = nc.dram_tensor("w1sb", [81, 64], dt.bfloat16, kind="ExternalInput")
    w2d = nc.dram_tensor("w2sb", [128, 10, 128], FP8, kind="ExternalInput")
    w3d = nc.dram_tensor("w3sb", [128, 6, 128], FP8, kind="ExternalInput")
    w4d = nc.dram_tensor("w4sb", [128, 10, 128], FP8, kind="ExternalInput")
    w5d = nc.dram_tensor("w5sb", [128, 10, 256], FP8, kind="ExternalInput")
    w6d = nc.dram_tensor("w6sb", [128, 2, 10, 256], FP8, kind="ExternalInput")
    sbd = {}
    sbshape = {1: [128, 1, 2], 2: [128, 1, 2], 3: [128, 1, 2],
               4: [128, 1, 2], 5: [128, 2, 2], 6: [128, 2, 2]}
    for i in range(1, 7):
        sbd[i] = nc.dram_tensor(f"sb{i}", sbshape[i], dt.float32,
                                kind="ExternalInput")
    wf1d = nc.dram_tensor("wf1sb", [128, 2 * 16 * 512], FP8,
                          kind="ExternalInput")
    sbf1d = nc.dram_tensor("sbf1", [128, 4, 2], dt.float32,
                           kind="ExternalInput")
    wf2d = nc.dram_tensor("wf2sb", [128, 40], FP8, kind="ExternalInput")
    sbf2d = nc.dram_tensor("sbf2", [10, 2], dt.float32, kind="ExternalInput")
    outd = nc.dram_tensor("out", [B, 10], dt.float32, kind="ExternalOutput")

    with tile.TileContext(nc) as tc:
        # ------------- persistent weights ---------------------------------
        wp_cm = tc.tile_pool(name="weights", bufs=1)
        wp = wp_cm.__enter__()
        w1 = wp.tile([81, 64], dt.bfloat16, tag="w1")
        nc.sync.dma_start(w1[:], w1d[:])
        wsb = {}
        for i, shape in ((2, [128, 10, 128]), (3, [128, 6, 128]),
                         (4, [128, 10, 128]), (5, [128, 10, 256]),
                         (6, [128, 2, 10, 256])):
            t = wp.tile(shape, FP8, tag=f"w{i}")
            wsb[i] = t
        # w2 is needed right after L1; the rest stream in behind the xcol
        # chunks (their dma_starts are issued after the L1 loop below)
        nc.scalar.dma_start(wsb[2][:], w2d[:])
        sbt = {}
        sbt0 = {}
        for i in range(1, 7):
            t0 = wp.tile(sbshape[i], dt.float32, tag=f"s{i}d")
            sbt0[i] = t0
            t = wp.tile(sbshape[i], dt.float32, tag=f"s{i}")
            sbt[i] = t
        nc.sync.dma_start(sbt0[1][:], sbd[1][:])
        nc.scalar.copy(sbt[1][:], sbt0[1][:])
        wf2 = wp.tile([128, 40], FP8, tag="wf2")
        sbf2t = wp.tile([10, 2], dt.float32, tag="sf2d")
        sbf2 = wp.tile([10, 2], dt.float32, tag="sf2")

        def zero_border(A, Hp, eng=None):
            e = eng or nc.gpsimd
            e.memset(A[:, 0, :, :], 0.0)
            e.memset(A[:, Hp - 1, :, :], 0.0)
            e.memset(A[:, 1:Hp - 1, 0, :], 0.0)
            e.memset(A[:, 1:Hp - 1, Hp - 1, :], 0.0)

        def with_pair(ap, delta):
            APc = type(ap)
            pairs = list(ap.ap)
            return APc(ap.tensor, ap.offset,
                       [pairs[0], [delta, 2]] + list(pairs[1:]))

        def dr_group(ps_ap, wt, co_sl, base_fn, deltas, npairs, extra=None):
            """Accumulate npairs DR matmuls (+ optional extra groups)."""
            for p in range(npairs):
                nc.tensor.matmul(ps_ap, wt[:, 2 * p:2 * p + 2, co_sl],
                                 with_pair(base_fn(p), deltas[p]),
                                 start=(p == 0), stop=(extra is None
                                                       and p == npairs - 1),
                                 perf_mode=DR)
            if extra is not None:
                wt2, base_fn2, deltas2 = extra
                for p in range(npairs):
                    nc.tensor.matmul(ps_ap, wt2[:, 2 * p:2 * p + 2, co_sl],
                                     with_pair(base_fn2(p), deltas2[p]),
                                     start=False, stop=(p == npairs - 1),
                                     perf_mode=DR)

        # ------------- Layer 1: K=81 im2col conv, x-halves in PSUM --------
        # PSUM partitions 0-63 = left 16 output cols, 64-127 = right 16.
        # The quantized row then writes A2's lower (channels, x) AND upper
        # (x+16 view) halves in ONE vector op; only two boundary columns
        # (upper col0 = real x15, lower col17 = real x16) need patch DMAs.
        pa2_cm = tc.tile_pool(name="A2", bufs=1)
        pa2 = pa2_cm.__enter__()
        A2 = pa2.tile([128, 34, 18, B], FP8, tag="A2")
        nc.vector.memset(A2[:, 0, :, :], 0.0)
        nc.vector.memset(A2[:, 33, :, :], 0.0)
        nc.vector.memset(A2[:, 1:33, 0, :], 0.0)
        nc.vector.memset(A2[:, 1:33, 17, :], 0.0)
        pxc_cm = tc.tile_pool(name="l1mov", bufs=16)
        pxc = pxc_cm.__enter__()
        dma_engs = (nc.sync, nc.scalar, nc.gpsimd)
        movs = []
        with (tc.tile_pool(name="l1ps", bufs=2, space="PSUM") as pps,
              tc.tile_pool(name="l1z", bufs=2) as pz):
            for q in range(16):  # y-pairs; per-row transfers, 3 queues
                for yy in range(2):
                    y = 2 * q + yy
                    mov = pxc.tile([81, 32, B], dt.bfloat16, tag="mov")
                    dma_engs[y % 3].dma_start(mov[:], xcold[:, y, :, :])
                    movs.append(mov)
                ps = pps.tile([128, 4, 512], dt.float32, tag="ps")
                for yy in range(2):
                    mov = movs[2 * q + yy]
                    for g in range(2):
                        nc.tensor.matmul(ps[0:64, 2 * yy + g, :], w1[:],
                                         mov[:, g * 8:g * 8 + 8, :],
                                         start=True, stop=True)
                        nc.tensor.matmul(ps[64:128, 2 * yy + g, :], w1[:],
                                         mov[:, 16 + g * 8:
                                              16 + g * 8 + 8, :],
                                         start=True, stop=True)
                z7 = pz.tile([128, 4, 512], dt.float32, tag="z")
                nc.scalar.activation(z7[:], ps[:], AF.Relu,
                                     bias=sbt[1][:, 0, 1:2],
                                     scale=sbt[1][:, 0, 0:1])
                zf = z7[:].rearrange("p a b -> p (a b)")
                nc.vector.tensor_scalar(zf, zf, MAGIC, MAGIC, OP.add, OP.max)
                for yy in range(2):
                    nc.vector.tensor_scalar(
                        A2[:, 1 + 2 * q + yy, 1:17, :].rearrange(
                            "p x b -> p (x b)"),
                        z7[:, 2 * yy:2 * yy + 2, :].rearrange(
                            "p g b -> p (g b)"),
                        MAGIC, 7.0, OP.subtract, OP.min)
        # boundary patch columns (after all xcol chunks: keep queues clean)
        for k in range(4):
            r0 = 1 + 8 * k
            nc.gpsimd.dma_start(A2[64:128, r0:r0 + 8, 0:1, :],
                                A2[0:64, r0:r0 + 8, 16:17, :])
            nc.gpsimd.dma_start(A2[0:64, r0:r0 + 8, 17:18, :],
                                A2[64:128, r0:r0 + 8, 1:2, :])
        # stream the remaining weights behind the L1 loads
        nc.scalar.dma_start(wsb[3][:], w3d[:])
        nc.scalar.dma_start(wsb[4][:], w4d[:])
        nc.sync.dma_start(wsb[5][:], w5d[:])
        nc.sync.dma_start(wsb[6][:], w6d[:])
        for i in range(2, 7):
            nc.sync.dma_start(sbt0[i][:], sbd[i][:])
            nc.scalar.copy(sbt[i][:], sbt0[i][:])
        nc.sync.dma_start(wf2[:], wf2d[:])
        nc.sync.dma_start(sbf2t[:], sbf2d[:])
        nc.scalar.copy(sbf2[:], sbf2t[:])
        pxc_cm.__exit__(None, None, None)  # free the xcol staging space
        fcw_cm = tc.tile_pool(name="fcw", bufs=1)
        fcw = fcw_cm.__enter__()
        wf1 = fcw.tile([128, 2 * 16 * 512], FP8, tag="wf1")
        sbf1t = fcw.tile([128, 4, 2], dt.float32, tag="sf1d")
        sbf1 = fcw.tile([128, 4, 2], dt.float32, tag="sf1")
        nc.scalar.dma_start(sbf1t[:], sbf1d[:])
        nc.scalar.copy(sbf1[:], sbf1t[:])

        # ------------- Layer 2 (64ch 32x32, halves, pool -> 16) -----------
        pa3_cm = tc.tile_pool(name="A3", bufs=1)
        pa3 = pa3_cm.__enter__()
        A3 = pa3.tile([128, 18, 18, B], FP8, tag="A3")
        zero_border(A3, 18)
        d2 = _pair_deltas(18)
        with (tc.tile_pool(name="c2ps", bufs=3, space="PSUM") as pps,
              tc.tile_pool(name="c2z", bufs=2) as pz,
              tc.tile_pool(name="c2t", bufs=2) as pt):
            w2, sb2 = wsb[2], sbt[2]
            for yo in range(16):
                z7 = pz.tile([128, 2, 2, 8, B], dt.float32, tag="z")
                for yy in range(2):
                    y = 2 * yo + yy
                    ps = pps.tile([128, 2, 512], dt.float32, tag="ps")
                    for xh in range(2):
                        x0 = 8 * xh

                        def mkbase(p, _y=y, _x0=x0):
                            dy, dx = TAPS[2 * p] if p < 4 else TAPS[8]
                            return A2[:, _y + dy, _x0 + dx:_x0 + dx + 8, :]
                        dr_group(ps[:, xh, :], w2, slice(0, 128), mkbase,
                                 d2, 5)
                    nc.scalar.activation(
                        z7[:, yy, :, :, :].rearrange("p a x b -> p (a x b)")
                        .rearrange("p (a b) -> p a b", b=512),
                        ps[:], AF.Relu, bias=sb2[:, 0, 1:2],
                        scale=sb2[:, 0, 0:1])
                zx = pt.tile([128, 2, 2, 4, B], dt.float32, tag="zx")
                for yy in range(2):
                    nc.vector.tensor_tensor(
                        zx[:, yy, :, :, :], z7[:, yy, :, 0::2, :],
                        z7[:, yy, :, 1::2, :], OP.max)
                zp = pt.tile([128, 2, 4, B], dt.float32, tag="zp")
                nc.vector.tensor_tensor(zp[:], zx[:, 0, :, :, :],
                                        zx[:, 1, :, :, :], OP.max)
                zpf = zp[:].rearrange("p a x b -> p (a x b)")
                nc.vector.tensor_scalar(zpf, zpf, MAGIC, MAGIC,
                                        OP.add, OP.max)
                a3t = pt.tile([128, 2, 4, B], FP8, tag="a3t")
                nc.vector.tensor_scalar(
                    a3t[:].rearrange("p a x b -> p (a x b)"), zpf, MAGIC, 7.0,
                    OP.subtract, OP.min)
                nc.sync.dma_start(A3[0:64, 1 + yo, 1:9, :],
                                  a3t[0:64].rearrange("p a x b -> p (a x) b"))
                nc.sync.dma_start(A3[0:64, 1 + yo, 9:17, :],
                                  a3t[64:128].rearrange(
                                      "p a x b -> p (a x) b"))
                # x+1 dup copy for L3 dx-packing, 4 chunks
                if yo in (2, 7, 12, 15):
                    r0, r1 = {2: (0, 4), 7: (4, 9), 12: (9, 14),
                              15: (14, 18)}[yo]
                    nc.gpsimd.dma_start(A3[64:128, r0:r1, 0:17, :],
                                        A3[0:64, r0:r1, 1:18, :])

        nc.gpsimd.dma_start(wf1[:], wf1d[:])

        # ------------- Layer 3 (64 -> 128, 16x16, dx-packed) --------------
        pa4_cm = tc.tile_pool(name="A4", bufs=1)
        pa4 = pa4_cm.__enter__()
        A4 = pa4.tile([128, 18, 18, B], FP8, tag="A4")
        zero_border(A4, 18)
        d3 = [2 * B, 2 * B, 2 * B]
        with (tc.tile_pool(name="c3ps", bufs=3, space="PSUM") as pps,
              tc.tile_pool(name="c3z", bufs=2) as pz):
            w3, sb3 = wsb[3], sbt[3]
            for y in range(16):
                ps = pps.tile([128, 2, 512], dt.float32, tag="ps")
                for xh in range(2):
                    x0 = 8 * xh
                    for dy in range(3):
                        base = A3[:, y + dy, x0:x0 + 8, :]
                        nc.tensor.matmul(
                            ps[:, xh, :], w3[:, 2 * dy:2 * dy + 2, :],
                            with_pair(base, 2 * B), start=(dy == 0),
                            stop=(dy == 2), perf_mode=DR)
                z7 = pz.tile([128, 2, 8, B], dt.float32, tag="z")
                nc.scalar.activation(
                    z7[:].rearrange("p a x b -> p (a x b)")
                    .rearrange("p (a b) -> p a b", b=512),
                    ps[:], AF.Relu, bias=sb3[:, 0, 1:2], scale=sb3[:, 0, 0:1])
                zf = z7[:].rearrange("p a x b -> p (a x b)")
                nc.vector.tensor_scalar(zf, zf, MAGIC, MAGIC, OP.add, OP.max)
                nc.vector.tensor_scalar(
                    A4[:, 1 + y, 1:17, :].rearrange("p x b -> p (x b)"),
                    zf, MAGIC, 7.0, OP.subtract, OP.min)

        # ------------- Layer 4 (128 -> 128, 16x16, pool -> 8) -------------
        pa5_cm = tc.tile_pool(name="A5", bufs=1)
        pa5 = pa5_cm.__enter__()
        A5 = pa5.tile([128, 10, 10, B], FP8, tag="A5")
        zero_border(A5, 10)
        d4 = _pair_deltas(18)
        with (tc.tile_pool(name="c4ps", bufs=3, space="PSUM") as pps,
              tc.tile_pool(name="c4z", bufs=2) as pz,
              tc.tile_pool(name="c4t", bufs=2) as pt):
            w4, sb4 = wsb[4], sbt[4]
            for yo in range(8):
                z7 = pz.tile([128, 2, 2, 8, B], dt.float32, tag="z")
                for yy in range(2):
                    y = 2 * yo + yy
                    ps = pps.tile([128, 2, 512], dt.float32, tag="ps")
                    for xh in range(2):
                        x0 = 8 * xh

                        def mkbase(p, _y=y, _x0=x0):
                            dy, dx = TAPS[2 * p] if p < 4 else TAPS[8]
                            return A4[:, _y + dy, _x0 + dx:_x0 + dx + 8, :]
                        dr_group(ps[:, xh, :], w4, slice(0, 128), mkbase,
                                 d4, 5)
                    nc.scalar.activation(
                        z7[:, yy, :, :, :].rearrange("p a x b -> p (a x b)")
                        .rearrange("p (a b) -> p a b", b=512),
                        ps[:], AF.Relu, bias=sb4[:, 0, 1:2],
                        scale=sb4[:, 0, 0:1])
                zx = pt.tile([128, 2, 2, 4, B], dt.float32, tag="zx")
                for yy in range(2):
                    nc.vector.tensor_tensor(
                        zx[:, yy, :, :, :], z7[:, yy, :, 0::2, :],
                        z7[:, yy, :, 1::2, :], OP.max)
                zp = pt.tile([128, 2, 4, B], dt.float32, tag="zp")
                nc.vector.tensor_tensor(zp[:], zx[:, 0, :, :, :],
                                        zx[:, 1, :, :, :], OP.max)
                zpf = zp[:].rearrange("p a x b -> p (a x b)")
                nc.vector.tensor_scalar(zpf, zpf, MAGIC, MAGIC,
                                        OP.add, OP.max)
                nc.vector.tensor_scalar(
                    A5[:, 1 + yo, 1:9, :].rearrange("p x b -> p (x b)"),
                    zpf, MAGIC, 7.0, OP.subtract, OP.min)

        # ------------- Layer 5 (128 -> 256, 8x8) --------------------------
        pa6_cm = tc.tile_pool(name="A6", bufs=1)
        pa6 = pa6_cm.__enter__()
        A6 = pa6.tile([128, 2, 10, 10, B], FP8, tag="A6")
        nc.gpsimd.memset(A6[:, :, 0, :, :], 0.0)
        nc.gpsimd.memset(A6[:, :, 9, :, :], 0.0)
        nc.gpsimd.memset(A6[:, :, 1:9, 0, :], 0.0)
        nc.gpsimd.memset(A6[:, :, 1:9, 9, :], 0.0)
        d5 = _pair_deltas(10)
        with (tc.tile_pool(name="c5ps", bufs=3, space="PSUM") as pps,
              tc.tile_pool(name="c5z", bufs=2) as pz):
            w5, sb5 = wsb[5], sbt[5]
            for y in range(8):
                ps = pps.tile([128, 2, 512], dt.float32, tag="ps")
                for ct in range(2):
                    def mkbase(p, _y=y):
                        dy, dx = TAPS[2 * p] if p < 4 else TAPS[8]
                        return A5[:, _y + dy, dx:dx + 8, :]
                    dr_group(ps[:, ct, :], w5,
                             slice(ct * 128, ct * 128 + 128), mkbase, d5, 5)
                z7 = pz.tile([128, 2, 8, B], dt.float32, tag="z")
                for ct in range(2):
                    nc.scalar.activation(
                        z7[:, ct, :, :].rearrange("p x b -> p (x b)"),
                        ps[:, ct, :],
                        AF.Relu, bias=sb5[:, ct, 1:2], scale=sb5[:, ct, 0:1])
                zf = z7[:].rearrange("p c x b -> p (c x b)")
                nc.vector.tensor_scalar(zf, zf, MAGIC, MAGIC, OP.add, OP.max)
                for ct in range(2):
                    nc.vector.tensor_scalar(
                        A6[:, ct, 1 + y, 1:9, :].rearrange(
                            "p x b -> p (x b)"),
                        z7[:, ct, :, :].rearrange("p x b -> p (x b)"),
                        MAGIC, 7.0, OP.subtract, OP.min)

        # ------------- Layer 6 (256 -> 256, 8x8, pool -> 4) ---------------
        pa7_cm = tc.tile_pool(name="A7", bufs=1)
        pa7 = pa7_cm.__enter__()
        A7 = pa7.tile([128, 2, 4, 4, B], FP8, tag="A7")  # unpadded, feeds FC
        d6 = _pair_deltas(10)
        with (tc.tile_pool(name="c6ps", bufs=3, space="PSUM") as pps,
              tc.tile_pool(name="c6z", bufs=2) as pz,
              tc.tile_pool(name="c6t", bufs=2) as pt):
            w6, sb6 = wsb[6], sbt[6]
            for yo in range(4):
                z7 = pz.tile([128, 2, 2, 8, B], dt.float32, tag="z")
                for yy in range(2):
                    y = 2 * yo + yy
                    ps = pps.tile([128, 2, 512], dt.float32, tag="ps")
                    for ct in range(2):
                        def mkb0(p, _y=y):
                            dy, dx = TAPS[2 * p] if p < 4 else TAPS[8]
                            return A6[:, 0, _y + dy, dx:dx + 8, :]

                        def mkb1(p, _y=y):
                            dy, dx = TAPS[2 * p] if p < 4 else TAPS[8]
                            return A6[:, 1, _y + dy, dx:dx + 8, :]
                        dr_group(ps[:, ct, :], w6[:, 0],
                                 slice(ct * 128, ct * 128 + 128), mkb0, d6, 5,
                                 extra=(w6[:, 1], mkb1, d6))
                    for ct in range(2):
                        nc.scalar.activation(
                            z7[:, yy, ct, :, :].rearrange(
                                "p x b -> p (x b)"),
                            ps[:, ct, :],
                            AF.Relu, bias=sb6[:, ct, 1:2],
                            scale=sb6[:, ct, 0:1])
                zx = pt.tile([128, 2, 2, 4, B], dt.float32, tag="zx")
                for yy in range(2):
                    nc.vector.tensor_tensor(
                        zx[:, yy, :, :, :], z7[:, yy, :, 0::2, :],
                        z7[:, yy, :, 1::2, :], OP.max)
                zp = pt.tile([128, 2, 4, B], dt.float32, tag="zp")
                nc.vector.tensor_tensor(zp[:], zx[:, 0, :, :, :],
                                        zx[:, 1, :, :, :], OP.max)
                zpf = zp[:].rearrange("p c x b -> p (c x b)")
                nc.vector.tensor_scalar(zpf, zpf, MAGIC, MAGIC,
                                        OP.add, OP.max)
                for ct in range(2):
                    nc.vector.tensor_scalar(
                        A7[:, ct, yo, :, :].rearrange("p x b -> p (x b)"),
                        zp[:, ct, :, :].rearrange("p x b -> p (x b)"),
                        MAGIC, 7.0, OP.subtract, OP.min)

        # ------------- FC1 (4096 -> 512) ----------------------------------
        pa8_cm = tc.tile_pool(name="A8", bufs=1)
        pa8 = pa8_cm.__enter__()
        A8 = pa8.tile([128, 4, B], FP8, tag="A8")
        with (tc.tile_pool(name="f1ps", bufs=4, space="PSUM") as pps,
              tc.tile_pool(name="f1t", bufs=4) as pt):
            for ct in range(4):
                ps = pps.tile([128, B], dt.float32, tag="ps")
                k = 0
                for cig in range(2):
                    for px in range(16):
                        wo = (cig * 16 + px) * 512 + ct * 128
                        nc.tensor.matmul(ps[:], wf1[:, wo:wo + 128],
                                         A7[:, cig, px // 4, px % 4, :],
                                         start=(k == 0), stop=(k == 31))
                        k += 1
                z7 = pt.tile([128, B], dt.float32, tag="z")
                nc.scalar.activation(z7[:], ps[:], AF.Relu,
                                     bias=sbf1[:, ct, 1:2],
                                     scale=sbf1[:, ct, 0:1])
                nc.vector.tensor_scalar(z7[:], z7[:], MAGIC, MAGIC,
                                        OP.add, OP.max)
                nc.vector.tensor_scalar(A8[:, ct, :], z7[:], MAGIC, 7.0,
                                        OP.subtract, OP.min)

        # ------------- FC2 (512 -> 10), signed output ---------------------
        with (tc.tile_pool(name="f2ps", bufs=1, space="PSUM") as pps,
              tc.tile_pool(name="f2t", bufs=1) as pt):
            ps = pps.tile([10, B], dt.float32, tag="ps")
            for kt in range(4):
                nc.tensor.matmul(ps[:], wf2[:, kt * 10:(kt + 1) * 10],
                                 A8[:, kt, :], start=(kt == 0), stop=(kt == 3))
            z7 = pt.tile([10, B], dt.float32, tag="z")
            nc.vector.tensor_scalar(z7[:], ps[:], sbf2[:, 0:1], sbf2[:, 1:2],
                                    OP.mult, OP.add)
            r = pt.tile([10, B], dt.float32, tag="r")
            nc.vector.tensor_scalar(r[:], z7[:], MAGIC, MAGIC,
                                    OP.add, OP.subtract)  # pure RNE
            r2 = pt.tile([10, B], dt.float32, tag="r2")
            nc.vector.tensor_scalar(r2[:], r[:], -7.0, 7.0, OP.max, OP.min)
            fin = pt.tile([10, B], dt.float32, tag="fin")
            nc.vector.tensor_scalar(fin[:], r2[:], 1.0 / 7.0,
                                    None, OP.mult)
            nc.sync.dma_start(outd[:].rearrange("b c -> c b"), fin[:])
        for cm in (pa8_cm, pa7_cm, pa6_cm, pa5_cm, pa4_cm, pa3_cm, fcw_cm,
                   pa2_cm):
            cm.__exit__(None, None, None)
        wp_cm.__exit__(None, None, None)

    nc.compile()
    return nc


# ----------------------------------------------------------------------------
# Entry point
# ----------------------------------------------------------------------------

_NC_CACHE = {}
LAST_RESULTS = None  # BassKernelResults of the most recent run (for test.py)


def kernel(**inputs):
    global LAST_RESULTS
    from concourse.bass_utils import run_bass_kernel_spmd
    if "nc" not in _NC_CACHE:
        _NC_CACHE["nc"] = build_nc()
    nc = _NC_CACHE["nc"]
    in_maps = host_pack(inputs)
    res = run_bass_kernel_spmd(nc, in_maps, list(range(N_CORES)))
    LAST_RESULTS = res
    outs = [res.results[c]["out"] for c in range(N_CORES)]
    return np.concatenate(outs, axis=0).astype(np.float32)


# revision 35
# speedup vs baseline: 1.3012x; 1.0082x over previous
"""Trainium2 Bass kernel for nn_IntegerCifar10Net (quantized VGG-ish CNN).

Data parallel over 8 NeuronCores, B=64 images/core.

v2: DoubleRow fp8 matmuls contract two conv taps per moving stream
(9 taps -> 5 DR matmuls), all matmuls full 128-column, activations in
batch-major padded SBUF layout [ci, y, x, b] fp8 integers {0..7} (=7*v).

Layer schemes:
  L1 : exact 3-plane bf16 im2col (K=81), co=64; left/right x-halves go to
       PSUM partitions 0-63 / 64-127 via concurrent column groups, so the
       quantized row writes A2's lower (channels) and upper (x+16 view)
       halves in one 128-lane vector op; two boundary columns are patched
       by tiny DMAs. A2 is stored 18 columns wide (only cols 0-17 are read).
  L2 : "halves" trick - PSUM partitions 0-63 = left 16 output cols,
       64-127 = right 16 (upper SBUF partitions hold an x+16 shifted copy);
       block-diagonal [W 0; 0 W] weights, 5 DR matmuls per bank.
  L3 : upper partitions hold x+1 shifted copy (dx-packing): 6 virtual taps
       -> 3 DR matmuls per bank, co=128.
  L4-L6: plain 9 (or 2x9) taps -> 5 (10) DR matmuls per bank.
  FC : as before (N=64 matmuls are dispatch-floor bound and cheap).

Quant chain per bank: ScalarE activation (relu, per-channel scale/bias),
VectorE max-pool (free-dim pairs), RNE round via +/-1.5*2^23 magic, clamp,
cast to fp8. Shift copies (x+16 / x+1) are chunked SBUF->SBUF DMAs
interleaved with compute rows so they stay off the critical path.
"""

import sys
import numpy as np

sys.path.insert(0, "/opt/trn_rl_repo")

import ml_dtypes

N_CORES = 8
B = 64  # images per core
MAGIC = 12582912.0  # 1.5 * 2^23 : RNE rounding magic for |v| < 2^22

# tap pair schedule for 3x3 convs: pairs of taps t=(dy,dx) row-major,
# 10th tap is zero-weight padding with moving delta -B (always in bounds)
TAPS = [(dy, dx) for dy in range(3) for dx in range(3)]


def _pair_deltas(W):
    """Moving-AP element deltas between the two taps of each DR pair."""
    ds = []
    for p in range(4):
        (dya, dxa), (dyb, dxb) = TAPS[2 * p], TAPS[2 * p + 1]
        ds.append(((dyb - dya) * W + (dxb - dxa)) * B)
    ds.append(-B)  # pad pair: (t8, zero-weight tap at x-1)
    return ds


# ----------------------------------------------------------------------------
# Host-side packing
# ----------------------------------------------------------------------------

def _qint(w):
    """round(clip(w,-1,1)*7) as float32 integers, matching jax fp32 chain."""
    w = np.asarray(w, np.float32)
    return np.round(np.clip(w, -1.0, 1.0) * np.float32(7.0)).astype(np.float32)


def _scale_bias(g, b, denom):
    # z7 = conv_int * (7*g/denom) + 7*b, constants in f64 then rounded to f32
    s = (7.0 * np.asarray(g, np.float64) / denom).astype(np.float32)
    bt = (7.0 * np.asarray(b, np.float64)).astype(np.float32)
    return np.ascontiguousarray(np.stack([s, bt], axis=1))  # [co, 2] f32


def _im2col_bf16x3(x):
    """x [B,3,32,32] f32 -> [81, 32, 32, B] bf16 (hi/mid/lo x 27 rows)."""
    Bc = x.shape[0]
    xp = np.zeros((Bc, 3, 34, 34), np.float32)
    xp[:, :, 1:33, 1:33] = x
    planes = np.empty((27, 32, 32, Bc), np.float32)
    k = 0
    for ci in range(3):
        for dy in range(3):
            for dx in range(3):
                planes[k] = np.transpose(xp[:, ci, dy:dy + 32, dx:dx + 32],
                                         (1, 2, 0))
                k += 1
    hi = planes.astype(ml_dtypes.bfloat16)
    r1 = planes - hi.astype(np.float32)
    mid = r1.astype(ml_dtypes.bfloat16)
    lo = (r1 - mid.astype(np.float32)).astype(ml_dtypes.bfloat16)
    return np.ascontiguousarray(np.concatenate([hi, mid, lo], axis=0))


def host_pack(inputs):
    """Build the per-core DRAM input dicts (weights replicated)."""
    f8 = ml_dtypes.float8_e4m3
    wc = {}
    # L1 weights: [64,3,3,3] -> lhsT [27,64], tripled to [81,64] bf16
    t = np.transpose(_qint(inputs["w1"]), (1, 2, 3, 0)).reshape(27, 64)
    wc["w1sb"] = np.ascontiguousarray(
        np.concatenate([t, t, t], axis=0).astype(ml_dtypes.bfloat16))
    sb = _scale_bias(inputs["g1"], inputs["b1"], 7.0)  # [64, 2]
    wc["sb1"] = np.ascontiguousarray(
        np.concatenate([sb, sb], axis=0).reshape(128, 1, 2))
    # L2: halves block-diagonal [128, 10, 128]
    wq = _qint(inputs["w2"])  # [64co, 64ci, 3, 3]
    w2p = np.zeros((128, 10, 128), np.float32)
    for ti, (dy, dx) in enumerate(TAPS):
        blk = wq[:, :, dy, dx].T  # [ci, co]
        w2p[0:64, ti, 0:64] = blk
        w2p[64:128, ti, 64:128] = blk
    wc["w2sb"] = np.ascontiguousarray(w2p.astype(f8))
    sb = _scale_bias(inputs["g2"], inputs["b2"], 49.0)
    wc["sb2"] = np.ascontiguousarray(
        np.concatenate([sb, sb], axis=0).reshape(128, 1, 2))
    # L3: dx-packed virtual taps [128, 6, 128]
    wq = _qint(inputs["w3"])  # [128co, 64ci, 3, 3]
    w3p = np.zeros((128, 6, 128), np.float32)
    for dy in range(3):
        for oi, o in enumerate((0, 2)):
            v = 2 * dy + oi
            w3p[0:64, v, :] = wq[:, :, dy, o].T
            if o == 0:
                w3p[64:128, v, :] = wq[:, :, dy, 1].T
    wc["w3sb"] = np.ascontiguousarray(w3p.astype(f8))
    wc["sb3"] = np.ascontiguousarray(
        _scale_bias(inputs["g3"], inputs["b3"], 49.0).reshape(128, 1, 2))
    # L4: [128, 10, 128]
    wq = _qint(inputs["w4"])  # [128, 128, 3, 3]
    w4p = np.zeros((128, 10, 128), np.float32)
    for ti, (dy, dx) in enumerate(TAPS):
        w4p[:, ti, :] = wq[:, :, dy, dx].T
    wc["w4sb"] = np.ascontiguousarray(w4p.astype(f8))
    wc["sb4"] = np.ascontiguousarray(
        _scale_bias(inputs["g4"], inputs["b4"], 49.0).reshape(128, 1, 2))
    # L5: [128, 10, 256]
    wq = _qint(inputs["w5"])  # [256, 128, 3, 3]
    w5p = np.zeros((128, 10, 256), np.float32)
    for ti, (dy, dx) in enumerate(TAPS):
        w5p[:, ti, :] = wq[:, :, dy, dx].T
    wc["w5sb"] = np.ascontiguousarray(w5p.astype(f8))
    wc["sb5"] = np.ascontiguousarray(
        _scale_bias(inputs["g5"], inputs["b5"], 49.0).reshape(2, 128, 2)
        .transpose(1, 0, 2))
    # L6: [128, 2, 10, 256] (cig-major)
    wq = _qint(inputs["w6"])  # [256, 256, 3, 3]
    w6p = np.zeros((128, 2, 10, 256), np.float32)
    for cig in range(2):
        for ti, (dy, dx) in enumerate(TAPS):
            w6p[:, cig, ti, :] = wq[:, cig * 128:(cig + 1) * 128, dy, dx].T
    wc["w6sb"] = np.ascontiguousarray(w6p.astype(f8))
    wc["sb6"] = np.ascontiguousarray(
        _scale_bias(inputs["g6"], inputs["b6"], 49.0).reshape(2, 128, 2)
        .transpose(1, 0, 2))
    # FC1 [512, 4096]: k=(c,y,x), c=cig*128+p  -> [128, (cig,16,512)]
    t = _qint(inputs["wf1"]).T.reshape(2, 128, 16, 512)
    wc["wf1sb"] = np.ascontiguousarray(
        np.transpose(t, (1, 0, 2, 3)).reshape(128, 2 * 16 * 512).astype(f8))
    wc["sbf1"] = np.ascontiguousarray(_scale_bias(
        inputs["gf1"], inputs["bf1"], 49.0).reshape(4, 128, 2).transpose(
        1, 0, 2))
    # FC2 [10, 512] -> [128, (4,10)]
    t = _qint(inputs["wf2"]).T.reshape(4, 128, 10)
    wc["wf2sb"] = np.ascontiguousarray(
        np.transpose(t, (1, 0, 2)).reshape(128, 40).astype(f8))
    wc["sbf2"] = _scale_bias(inputs["gf2"], inputs["bf2"], 49.0)

    x = np.asarray(inputs["x"], np.float32)
    maps = []
    for c in range(N_CORES):
        m = dict(wc)
        m["xcol"] = _im2col_bf16x3(x[c * B:(c + 1) * B])
        maps.append(m)
    return maps


# ----------------------------------------------------------------------------
# Bass program
# ----------------------------------------------------------------------------

def build_nc():
    import concourse.bacc as bacc
    import concourse.mybir as mybir
    import concourse.tile as tile

    dt = mybir.dt
    AF = mybir.ActivationFunctionType
    OP = mybir.AluOpType
    FP8 = dt.float8e4
    DR = mybir.MatmulPerfMode.DoubleRow

    nc = bacc.Bacc("TRN2", target_bir_lowering=False, debug=False)

    xcold = nc.dram_tensor("xcol", [81, 32, 32, B], dt.bfloat16,
                           kind="ExternalInput")
    w1d = nc.dram_tensor("w1sb", [81, 64], dt.bfloat16, kind="ExternalInput")
    w2d = nc.dram_tensor("w2sb", [128, 10, 128], FP8, kind="ExternalInput")
    w3d = nc.dram_tensor("w3sb", [128, 6, 128], FP8, kind="ExternalInput")
    w4d = nc.dram_tensor("w4sb", [128, 10, 128], FP8, kind="ExternalInput")
    w5d = nc.dram_tensor("w5sb", [128, 10, 256], FP8, kind="ExternalInput")
    w6d = nc.dram_tensor("w6sb", [128, 2, 10, 256], FP8, kind="ExternalInput")
    sbd = {}
    sbshape = {1: [128, 1, 2], 2: [128, 1, 2], 3: [128, 1, 2],
               4: [128, 1, 2], 5: [128, 2, 2], 6: [128, 2, 2]}
    for i in range(1, 7):
        sbd[i] = nc.dram_tensor(f"sb{i}", sbshape[i], dt.float32,
                                kind="ExternalInput")
    wf1d = nc.dram_tensor("wf1sb", [128, 2 * 16 * 512], FP8,
                          kind="ExternalInput")
    sbf1d = nc.dram_tensor("sbf1", [128, 4, 2], dt.float32,
                           kind="ExternalInput")
    wf2d = nc.dram_tensor("wf2sb", [128, 40], FP8, kind="ExternalInput")
    sbf2d = nc.dram_tensor("sbf2", [10, 2], dt.float32, kind="ExternalInput")
    outd = nc.dram_tensor("out", [B, 10], dt.float32, kind="ExternalOutput")

    with tile.TileContext(nc) as tc:
        # ------------- persistent weights ---------------------------------
        wp_cm = tc.tile_pool(name="weights", bufs=1)
        wp = wp_cm.__enter__()
        w1 = wp.tile([81, 64], dt.bfloat16, tag="w1")
        nc.sync.dma_start(w1[:], w1d[:])
        wsb = {}
        for i, shape in ((2, [128, 10, 128]), (3, [128, 6, 128]),
                         (4, [128, 10, 128]), (5, [128, 10, 256]),
                         (6, [128, 2, 10, 256])):
            t = wp.tile(shape, FP8, tag=f"w{i}")
            wsb[i] = t
        # w2 is needed right after L1; the rest stream in behind the xcol
        # chunks (their dma_starts are issued after the L1 loop below)
        nc.scalar.dma_start(wsb[2][:], w2d[:])
        sbt = {}
        sbt0 = {}
        for i in range(1, 7):
            t0 = wp.tile(sbshape[i], dt.float32, tag=f"s{i}d")
            sbt0[i] = t0
            t = wp.tile(sbshape[i], dt.float32, tag=f"s{i}")
            sbt[i] = t
        nc.sync.dma_start(sbt0[1][:], sbd[1][:])
        nc.scalar.copy(sbt[1][:], sbt0[1][:])
        wf2 = wp.tile([128, 40], FP8, tag="wf2")
        sbf2t = wp.tile([10, 2], dt.float32, tag="sf2d")
        sbf2 = wp.tile([10, 2], dt.float32, tag="sf2")

        def zero_border(A, Hp, eng=None):
            e = eng or nc.gpsimd
            e.memset(A[:, 0, :, :], 0.0)
            e.memset(A[:, Hp - 1, :, :], 0.0)
            e.memset(A[:, 1:Hp - 1, 0, :], 0.0)
            e.memset(A[:, 1:Hp - 1, Hp - 1, :], 0.0)

        def with_pair(ap, delta):
            APc = type(ap)
            pairs = list(ap.ap)
            return APc(ap.tensor, ap.offset,
                       [pairs[0], [delta, 2]] + list(pairs[1:]))

        def dr_group(ps_ap, wt, co_sl, base_fn, deltas, npairs, extra=None):
            """Accumulate npairs DR matmuls (+ optional extra groups)."""
            for p in range(npairs):
                nc.tensor.matmul(ps_ap, wt[:, 2 * p:2 * p + 2, co_sl],
                                 with_pair(base_fn(p), deltas[p]),
                                 start=(p == 0), stop=(extra is None
                                                       and p == npairs - 1),
                                 perf_mode=DR)
            if extra is not None:
                wt2, base_fn2, deltas2 = extra
                for p in range(npairs):
                    nc.tensor.matmul(ps_ap, wt2[:, 2 * p:2 * p + 2, co_sl],
                                     with_pair(base_fn2(p), deltas2[p]),
                                     start=False, stop=(p == npairs - 1),
                                     perf_mode=DR)

        # ------------- Layer 1: K=81 im2col conv, x-halves in PSUM --------
        # PSUM partitions 0-63 = left 16 output cols, 64-127 = right 16.
        # The quantized row then writes A2's lower (channels, x) AND upper
        # (x+16 view) halves in ONE vector op; only two boundary columns
        # (upper col0 = real x15, lower col17 = real x16) need patch DMAs.
        pa2_cm = tc.tile_pool(name="A2", bufs=1)
        pa2 = pa2_cm.__enter__()
        A2 = pa2.tile([128, 34, 18, B], FP8, tag="A2")
        nc.vector.memset(A2[:, 0, :, :], 0.0)
        nc.vector.memset(A2[:, 33, :, :], 0.0)
        nc.vector.memset(A2[:, 1:33, 0, :], 0.0)
        nc.vector.memset(A2[:, 1:33, 17, :], 0.0)
        pxc_cm = tc.tile_pool(name="l1mov", bufs=16)
        pxc = pxc_cm.__enter__()
        dma_engs = (nc.sync, nc.scalar, nc.gpsimd)
        movs = []
        with (tc.tile_pool(name="l1ps", bufs=2, space="PSUM") as pps,
              tc.tile_pool(name="l1z", bufs=2) as pz):
            for q in range(16):  # y-pairs; per-row transfers, 3 queues
                for yy in range(2):
                    y = 2 * q + yy
                    mov = pxc.tile([81, 32, B], dt.bfloat16, tag="mov")
                    dma_engs[y % 3].dma_start(mov[:], xcold[:, y, :, :])
                    movs.append(mov)
                ps = pps.tile([128, 4, 512], dt.float32, tag="ps")
                for yy in range(2):
                    mov = movs[2 * q + yy]
                    for g in range(2):
                        nc.tensor.matmul(ps[0:64, 2 * yy + g, :], w1[:],
                                         mov[:, g * 8:g * 8 + 8, :],
                                         start=True, stop=True)
                        nc.tensor.matmul(ps[64:128, 2 * yy + g, :], w1[:],
                                         mov[:, 16 + g * 8:
                                              16 + g * 8 + 8, :],
                                         start=True, stop=True)
                z7 = pz.tile([128, 4, 512], dt.float32, tag="z")
                nc.scalar.activation(z7[:], ps[:], AF.Relu,
                                     bias=sbt[1][:, 0, 1:2],
                                     scale=sbt[1][:, 0, 0:1])
                zf = z7[:].rearrange("p a b -> p (a b)")
                nc.vector.tensor_scalar(zf, zf, MAGIC, MAGIC, OP.add, OP.max)
                for yy in range(2):
                    nc.vector.tensor_scalar(
                        A2[:, 1 + 2 * q + yy, 1:17, :].rearrange(
                            "p x b -> p (x b)"),
                        z7[:, 2 * yy:2 * yy + 2, :].rearrange(
                            "p g b -> p (g b)"),
                        MAGIC, 7.0, OP.subtract, OP.min)
        # boundary patch columns (after all xcol chunks: keep queues clean)
        for k in range(4):
            r0 = 1 + 8 * k
            nc.gpsimd.dma_start(A2[64:128, r0:r0 + 8, 0:1, :],
                                A2[0:64, r0:r0 + 8, 16:17, :])
            nc.gpsimd.dma_start(A2[0:64, r0:r0 + 8, 17:18, :],
                                A2[64:128, r0:r0 + 8, 1:2, :])
        # stream the remaining weights behind the L1 loads
        nc.scalar.dma_start(wsb[3][:], w3d[:])
        nc.scalar.dma_start(wsb[4][:], w4d[:])
        nc.sync.dma_start(wsb[5][:], w5d[:])
        nc.sync.dma_start(wsb[6][:], w6d[:])
        for i in range(2, 7):
            nc.sync.dma_start(sbt0[i][:], sbd[i][:])
            nc.scalar.copy(sbt[i][:], sbt0[i][:])
        nc.sync.dma_start(wf2[:], wf2d[:])
        nc.sync.dma_start(sbf2t[:], sbf2d[:])
        nc.scalar.copy(sbf2[:], sbf2t[:])
        pxc_cm.__exit__(None, None, None)  # free the xcol staging space
        fcw_cm = tc.tile_pool(name="fcw", bufs=1)
        fcw = fcw_cm.__enter__()
        wf1 = fcw.tile([128, 2 * 16 * 512], FP8, tag="wf1")
        sbf1t = fcw.tile([128, 4, 2], dt.float32, tag="sf1d")
        sbf1 = fcw.tile([128, 4, 2], dt.float32, tag="sf1")
        nc.scalar.dma_start(sbf1t[:], sbf1d[:])
        nc.scalar.copy(sbf1[:], sbf1t[:])

        # ------------- Layer 2 (64ch 32x32, halves, pool -> 16) -----------
        pa3_cm = tc.tile_pool(name="A3", bufs=1)
        pa3 = pa3_cm.__enter__()
        A3 = pa3.tile([128, 18, 18, B], FP8, tag="A3")
        zero_border(A3, 18)
        d2 = _pair_deltas(18)
        with (tc.tile_pool(name="c2ps", bufs=3, space="PSUM") as pps,
              tc.tile_pool(name="c2z", bufs=2) as pz,
              tc.tile_pool(name="c2t", bufs=2) as pt):
            w2, sb2 = wsb[2], sbt[2]
            for yo in range(16):
                z7 = pz.tile([128, 2, 2, 8, B], dt.float32, tag="z")
                for yy in range(2):
                    y = 2 * yo + yy
                    ps = pps.tile([128, 2, 512], dt.float32, tag="ps")
                    for xh in range(2):
                        x0 = 8 * xh

                        def mkbase(p, _y=y, _x0=x0):
                            dy, dx = TAPS[2 * p] if p < 4 else TAPS[8]
                            return A2[:, _y + dy, _x0 + dx:_x0 + dx + 8, :]
                        dr_group(ps[:, xh, :], w2, slice(0, 128), mkbase,
                                 d2, 5)
                    nc.scalar.activation(
                        z7[:, yy, :, :, :].rearrange("p a x b -> p (a x b)")
                        .rearrange("p (a b) -> p a b", b=512),
                        ps[:], AF.Relu, bias=sb2[:, 0, 1:2],
                        scale=sb2[:, 0, 0:1])
                zx = pt.tile([128, 2, 2, 4, B], dt.float32, tag="zx")
                for yy in range(2):
                    nc.vector.tensor_tensor(
                        zx[:, yy, :, :, :], z7[:, yy, :, 0::2, :],
                        z7[:, yy, :, 1::2, :], OP.max)
                zp = pt.tile([128, 2, 4, B], dt.float32, tag="zp")
                nc.vector.tensor_tensor(zp[:], zx[:, 0, :, :, :],
                                        zx[:, 1, :, :, :], OP.max)
                zpf = zp[:].rearrange("p a x b -> p (a x b)")
                nc.vector.tensor_scalar(zpf, zpf, MAGIC, MAGIC,
                                        OP.add, OP.max)
                a3t = pt.tile([128, 2, 4, B], FP8, tag="a3t")
                nc.vector.tensor_scalar(
                    a3t[:].rearrange("p a x b -> p (a x b)"), zpf, MAGIC, 7.0,
                    OP.subtract, OP.min)
                nc.sync.dma_start(A3[0:64, 1 + yo, 1:9, :],
                                  a3t[0:64].rearrange("p a x b -> p (a x) b"))
                nc.sync.dma_start(A3[0:64, 1 + yo, 9:17, :],
                                  a3t[64:128].rearrange(
                                      "p a x b -> p (a x) b"))
                # x+1 dup copy for L3 dx-packing, 4 chunks
                if yo in (2, 7, 12, 15):
                    r0, r1 = {2: (0, 4), 7: (4, 9), 12: (9, 14),
                              15: (14, 18)}[yo]
                    nc.sync.dma_start(A3[64:128, r0:r1, 0:17, :],
                                      A3[0:64, r0:r1, 1:18, :])

        nc.gpsimd.dma_start(wf1[:], wf1d[:])

        # ------------- Layer 3 (64 -> 128, 16x16, dx-packed) --------------
        pa4_cm = tc.tile_pool(name="A4", bufs=1)
        pa4 = pa4_cm.__enter__()
        A4 = pa4.tile([128, 18, 18, B], FP8, tag="A4")
        zero_border(A4, 18)
        d3 = [2 * B, 2 * B, 2 * B]
        with (tc.tile_pool(name="c3ps", bufs=3, space="PSUM") as pps,
              tc.tile_pool(name="c3z", bufs=2) as pz):
            w3, sb3 = wsb[3], sbt[3]
            for y in range(16):
                ps = pps.tile([128, 2, 512], dt.float32, tag="ps")
                for xh in range(2):
                    x0 = 8 * xh
                    for dy in range(3):
                        base = A3[:, y + dy, x0:x0 + 8, :]
                        nc.tensor.matmul(
                            ps[:, xh, :], w3[:, 2 * dy:2 * dy + 2, :],
                            with_pair(base, 2 * B), start=(dy == 0),
                            stop=(dy == 2), perf_mode=DR)
                z7 = pz.tile([128, 2, 8, B], dt.float32, tag="z")
                nc.scalar.activation(
                    z7[:].rearrange("p a x b -> p (a x b)")
                    .rearrange("p (a b) -> p a b", b=512),
                    ps[:], AF.Relu, bias=sb3[:, 0, 1:2], scale=sb3[:, 0, 0:1])
                zf = z7[:].rearrange("p a x b -> p (a x b)")
                nc.vector.tensor_scalar(zf, zf, MAGIC, MAGIC, OP.add, OP.max)
                nc.vector.tensor_scalar(
                    A4[:, 1 + y, 1:17, :].rearrange("p x b -> p (x b)"),
                    zf, MAGIC, 7.0, OP.subtract, OP.min)

        # ------------- Layer 4 (128 -> 128, 16x16, pool -> 8) -------------
        pa5_cm = tc.tile_pool(name="A5", bufs=1)
        pa5 = pa5_cm.__enter__()
        A5 = pa5.tile([128, 10, 10, B], FP8, tag="A5")
        zero_border(A5, 10)
        d4 = _pair_deltas(18)
        with (tc.tile_pool(name="c4ps", bufs=3, space="PSUM") as pps,
              tc.tile_pool(name="c4z", bufs=2) as pz,
              tc.tile_pool(name="c4t", bufs=2) as pt):
            w4, sb4 = wsb[4], sbt[4]
            for yo in range(8):
                z7 = pz.tile([128, 2, 2, 8, B], dt.float32, tag="z")
                for yy in range(2):
                    y = 2 * yo + yy
                    ps = pps.tile([128, 2, 512], dt.float32, tag="ps")
                    for xh in range(2):
                        x0 = 8 * xh

                        def mkbase(p, _y=y, _x0=x0):
                            dy, dx = TAPS[2 * p] if p < 4 else TAPS[8]
                            return A4[:, _y + dy, _x0 + dx:_x0 + dx + 8, :]
                        dr_group(ps[:, xh, :], w4, slice(0, 128), mkbase,
                                 d4, 5)
                    nc.scalar.activation(
                        z7[:, yy, :, :, :].rearrange("p a x b -> p (a x b)")
                        .rearrange("p (a b) -> p a b", b=512),
                        ps[:], AF.Relu, bias=sb4[:, 0, 1:2],
                        scale=sb4[:, 0, 0:1])
                zx = pt.tile([128, 2, 2, 4, B], dt.float32, tag="zx")
                for yy in range(2):
                    nc.vector.tensor_tensor(
                        zx[:, yy, :, :, :], z7[:, yy, :, 0::2, :],
                        z7[:, yy, :, 1::2, :], OP.max)
                zp = pt.tile([128, 2, 4, B], dt.float32, tag="zp")
                nc.vector.tensor_tensor(zp[:], zx[:, 0, :, :, :],
                                        zx[:, 1, :, :, :], OP.max)
                zpf = zp[:].rearrange("p a x b -> p (a x b)")
                nc.vector.tensor_scalar(zpf, zpf, MAGIC, MAGIC,
                                        OP.add, OP.max)
                nc.vector.tensor_scalar(
                    A5[:, 1 + yo, 1:9, :].rearrange("p x b -> p (x b)"),
                    zpf, MAGIC, 7.0, OP.subtract, OP.min)

        # ------------- Layer 5 (128 -> 256, 8x8) --------------------------
        pa6_cm = tc.tile_pool(name="A6", bufs=1)
        pa6 = pa6_cm.__enter__()
        A6 = pa6.tile([128, 2, 10, 10, B], FP8, tag="A6")
        nc.gpsimd.memset(A6[:, :, 0, :, :], 0.0)
        nc.gpsimd.memset(A6[:, :, 9, :, :], 0.0)
        nc.gpsimd.memset(A6[:, :, 1:9, 0, :], 0.0)
        nc.gpsimd.memset(A6[:, :, 1:9, 9, :], 0.0)
        d5 = _pair_deltas(10)
        with (tc.tile_pool(name="c5ps", bufs=3, space="PSUM") as pps,
              tc.tile_pool(name="c5z", bufs=2) as pz):
            w5, sb5 = wsb[5], sbt[5]
            for y in range(8):
                ps = pps.tile([128, 2, 512], dt.float32, tag="ps")
                for ct in range(2):
                    def mkbase(p, _y=y):
                        dy, dx = TAPS[2 * p] if p < 4 else TAPS[8]
                        return A5[:, _y + dy, dx:dx + 8, :]
                    dr_group(ps[:, ct, :], w5,
                             slice(ct * 128, ct * 128 + 128), mkbase, d5, 5)
                z7 = pz.tile([128, 2, 8, B], dt.float32, tag="z")
                for ct in range(2):
                    nc.scalar.activation(
                        z7[:, ct, :, :].rearrange("p x b -> p (x b)"),
                        ps[:, ct, :],
                        AF.Relu, bias=sb5[:, ct, 1:2], scale=sb5[:, ct, 0:1])
                zf = z7[:].rearrange("p c x b -> p (c x b)")
                nc.vector.tensor_scalar(zf, zf, MAGIC, MAGIC, OP.add, OP.max)
                for ct in range(2):
                    nc.vector.tensor_scalar(
                        A6[:, ct, 1 + y, 1:9, :].rearrange(
                            "p x b -> p (x b)"),
                        z7[:, ct, :, :].rearrange("p x b -> p (x b)"),
                        MAGIC, 7.0, OP.subtract, OP.min)

        # ------------- Layer 6 (256 -> 256, 8x8, pool -> 4) ---------------
        pa7_cm = tc.tile_pool(name="A7", bufs=1)
        pa7 = pa7_cm.__enter__()
        A7 = pa7.tile([128, 2, 4, 4, B], FP8, tag="A7")  # unpadded, feeds FC
        d6 = _pair_deltas(10)
        with (tc.tile_pool(name="c6ps", bufs=3, space="PSUM") as pps,
              tc.tile_pool(name="c6z", bufs=2) as pz,
              tc.tile_pool(name="c6t", bufs=2) as pt):
            w6, sb6 = wsb[6], sbt[6]
            for yo in range(4):
                z7 = pz.tile([128, 2, 2, 8, B], dt.float32, tag="z")
                for yy in range(2):
                    y = 2 * yo + yy
                    ps = pps.tile([128, 2, 512], dt.float32, tag="ps")
                    for ct in range(2):
                        def mkb0(p, _y=y):
                            dy, dx = TAPS[2 * p] if p < 4 else TAPS[8]
                            return A6[:, 0, _y + dy, dx:dx + 8, :]

                        def mkb1(p, _y=y):
                            dy, dx = TAPS[2 * p] if p < 4 else TAPS[8]
                            return A6[:, 1, _y + dy, dx:dx + 8, :]
                        dr_group(ps[:, ct, :], w6[:, 0],
                                 slice(ct * 128, ct * 128 + 128), mkb0, d6, 5,
                                 extra=(w6[:, 1], mkb1, d6))
                    for ct in range(2):
                        nc.scalar.activation(
                            z7[:, yy, ct, :, :].rearrange(
                                "p x b -> p (x b)"),
                            ps[:, ct, :],
                            AF.Relu, bias=sb6[:, ct, 1:2],
                            scale=sb6[:, ct, 0:1])
                zx = pt.tile([128, 2, 2, 4, B], dt.float32, tag="zx")
                for yy in range(2):
                    nc.vector.tensor_tensor(
                        zx[:, yy, :, :, :], z7[:, yy, :, 0::2, :],
                        z7[:, yy, :, 1::2, :], OP.max)
                zp = pt.tile([128, 2, 4, B], dt.float32, tag="zp")
                nc.vector.tensor_tensor(zp[:], zx[:, 0, :, :, :],
                                        zx[:, 1, :, :, :], OP.max)
                zpf = zp[:].rearrange("p c x b -> p (c x b)")
                nc.vector.tensor_scalar(zpf, zpf, MAGIC, MAGIC,
                                        OP.add, OP.max)
                for ct in range(2):
                    nc.vector.tensor_scalar(
                        A7[:, ct, yo, :, :].rearrange("p x b -> p (x b)"),
                        zp[:, ct, :, :].rearrange("p x b -> p (x b)"),
                        MAGIC, 7.0, OP.subtract, OP.min)

        # ------------- FC1 (4096 -> 512) ----------------------------------
        pa8_cm = tc.tile_pool(name="A8", bufs=1)
        pa8 = pa8_cm.__enter__()
        A8 = pa8.tile([128, 4, B], FP8, tag="A8")
        with (tc.tile_pool(name="f1ps", bufs=4, space="PSUM") as pps,
              tc.tile_pool(name="f1t", bufs=4) as pt):
            for ct in range(4):
                ps = pps.tile([128, B], dt.float32, tag="ps")
                k = 0
                for cig in range(2):
                    for px in range(16):
                        wo = (cig * 16 + px) * 512 + ct * 128
                        nc.tensor.matmul(ps[:], wf1[:, wo:wo + 128],
                                         A7[:, cig, px // 4, px % 4, :],
                                         start=(k == 0), stop=(k == 31))
                        k += 1
                z7 = pt.tile([128, B], dt.float32, tag="z")
                nc.scalar.activation(z7[:], ps[:], AF.Relu,
                                     bias=sbf1[:, ct, 1:2],
                                     scale=sbf1[:, ct, 0:1])
                nc.vector.tensor_scalar(z7[:], z7[:], MAGIC, MAGIC,
                                        OP.add, OP.max)
                nc.vector.tensor_scalar(A8[:, ct, :], z7[:], MAGIC, 7.0,
                                        OP.subtract, OP.min)

        # ------------- FC2 (512 -> 10), signed output ---------------------
        with (tc.tile_pool(name="f2ps", bufs=1, space="PSUM") as pps,
              tc.tile_pool(name="f2t", bufs=1) as pt):
            ps = pps.tile([10, B], dt.float32, tag="ps")
            for kt in range(4):
                nc.tensor.matmul(ps[:], wf2[:, kt * 10:(kt + 1) * 10],
                                 A8[:, kt, :], start=(kt == 0), stop=(kt == 3))
            z7 = pt.tile([10, B], dt.float32, tag="z")
            nc.vector.tensor_scalar(z7[:], ps[:], sbf2[:, 0:1], sbf2[:, 1:2],
                                    OP.mult, OP.add)
            r = pt.tile([10, B], dt.float32, tag="r")
            nc.vector.tensor_scalar(r[:], z7[:], MAGIC, MAGIC,
                                    OP.add, OP.subtract)  # pure RNE
            r2 = pt.tile([10, B], dt.float32, tag="r2")
            nc.vector.tensor_scalar(r2[:], r[:], -7.0, 7.0, OP.max, OP.min)
            fin = pt.tile([10, B], dt.float32, tag="fin")
            nc.vector.tensor_scalar(fin[:], r2[:], 1.0 / 7.0,
                                    None, OP.mult)
            nc.sync.dma_start(outd[:].rearrange("b c -> c b"), fin[:])
        for cm in (pa8_cm, pa7_cm, pa6_cm, pa5_cm, pa4_cm, pa3_cm, fcw_cm,
                   pa2_cm):
            cm.__exit__(None, None, None)
        wp_cm.__exit__(None, None, None)

    nc.compile()
    return nc


# ----------------------------------------------------------------------------
# Entry point
# ----------------------------------------------------------------------------

_NC_CACHE = {}
LAST_RESULTS = None  # BassKernelResults of the most recent run (for test.py)


def kernel(**inputs):
    global LAST_RESULTS
    from concourse.bass_utils import run_bass_kernel_spmd
    if "nc" not in _NC_CACHE:
        _NC_CACHE["nc"] = build_nc()
    nc = _NC_CACHE["nc"]
    in_maps = host_pack(inputs)
    res = run_bass_kernel_spmd(nc, in_maps, list(range(N_CORES)))
    LAST_RESULTS = res
    outs = [res.results[c]["out"] for c in range(N_CORES)]
    return np.concatenate(outs, axis=0).astype(np.float32)
